# revision 43
# baseline (speedup 1.0000x reference)
"""Trainium2 Bass kernel for the Clopath plasticity rule (nn_Clopath).

Math (reference):
    dW_pot[b,e,o] = sum_d xbar[d,b,e] * dmap[d,e,o] * A_p[e,o] * gp[b,o]
    dW_dep[b,e,o] = sum_d Xd[d,b,e]   * dmap[d,e,o] * A_d[e,o] * gd[b,o]
        gp = Xpost * relu(u_pot),  gd = relu(u_dep)
    W_new = clip(W + dW_pot - dW_dep, 0, 2)
    out = W (pre-update);  plus three exponential trace updates.

Strategy (8 NeuronCores, shard pre-synaptic dim e; DMA-bound at ~161 us/core
per the instruction cost model, vs a ~149 us pure-bytes floor):
  * Host folds A into dmap:  DA_p = dmap*A_p, DA_d = dmap*A_d, stored as
    float8_e4m3 scaled by 256 (A ~ 1e-3 underflows e4m3; the 1/256
    compensation goes into the bf16 gate tensors - exact power of two).
    dmap is 0/1 so fp8 quantizes only A (~6% worst-case on a term that is
    ~1e-4 of W; measured absmax error 1.2e-4 relative on W_new).
  * The d-contraction runs on TensorE as block-diagonal packed matmuls:
    contraction K = 8 e's x 16 d = 128, out M = 64 = (b x e_local), N = 512
    o-chunk.  lhsT carries xbar (bf16) / Xd (fp8, exact 0/1) values on the
    block diagonal (built on host); rhs is a (128, 2048) repack of DA rows.
    Two matmuls (two e-halves) fill a (128, 512) PSUM tile at partition
    offsets 0/64.
  * Gating by gp/gd uses partition-replicated (128, N) bf16 tiles; ScalarE
    evacuates PSUM to bf16, VectorE does the gate multiplies (bf16 2x
    mode), pot-dep subtract, fp32 W add, and a fused min/max clip; W_new
    stores issue from the ACT HWDGE ring to overlap with SP-ring loads.
  * Trace updates run on-device as single fused scalar_tensor_tensor ops.
  * All elementwise/PE work hides under the DMA stream (W fp32 in+out
    33.6 MB/core + DA fp8 16.8 MB/core dominate).
"""

import numpy as np
import ml_dtypes

import concourse.bass as bass
import concourse.bacc as bacc
import concourse.mybir as mybir
from concourse.tile import TileContext
from concourse.bass_utils import run_bass_kernel_spmd

BF16 = ml_dtypes.bfloat16
F32 = np.float32

D, B, N = 16, 8, 2048
NCORES = 8
E = N // NCORES          # 256 pre-synaptic neurons per core
NSUP = E // 16           # 16 supers (16 e's each) per core
ALPHA_X, ALPHA_P, ALPHA_D = 0.95, 0.9, 0.8
WMAX = 2.0
OC = N // 512            # 4 o-chunks of 512


FP8_SCALE = 256.0


def build_kernel(repeat: int = 1, fp8_pot: bool = False, fp8_dep: bool = False,
                 rhs_bufs: int = 2, ev_bufs: int = 3, blob: bool = False,
                 ev_mode: str = 'default', dma_split: bool = False,
                 out_act: bool = False, w_act: bool = False,
                 psum_bufs: int = 3, lhsd_fp8: bool = False,
                 rd_act: bool = False, rd_gp: bool = False,
                 w_gp: bool = False, _tail_split: int = 16,
                 _skip: str = '') -> bass.Bass:
    if blob:
        assert fp8_pot and fp8_dep, "blob layout assumes fp8 DA tensors" 
    # Bacc (not plain Bass): its finalize() runs move_matmul_waits_to_
    # ldweights + generate_event_semaphores, which split multi-sem waits to
    # satisfy the 1-wait-per-instruction TRN2 codegen constraint.
    nc = bacc.Bacc()
    dt = mybir.dt
    f32, bf16 = dt.float32, dt.bfloat16
    dt_p = dt.float8e4 if fp8_pot else bf16
    dt_d = dt.float8e4 if fp8_dep else bf16

    # Per-core inputs (host pre-packed into DMA-friendly layouts).
    if blob:
        # One byte-interleaved tensor per super: per partition row =
        # [DA_p h0 | DA_p h1 | DA_d h0 | DA_d h1 (fp8, N bytes each) | W (f32, 4N bytes)]
        BLOB = nc.dram_tensor("BLOB", [NSUP, 128, 8 * N], dt.uint8,
                              kind="ExternalInput")
    else:
        DA_p = nc.dram_tensor("DA_p", [NSUP, 2, 128, N], dt_p, kind="ExternalInput")
        DA_d = nc.dram_tensor("DA_d", [NSUP, 2, 128, N], dt_d, kind="ExternalInput")
        W_in = nc.dram_tensor("W_in", [NSUP, 128, N], f32, kind="ExternalInput")
    LHS_p = nc.dram_tensor("LHS_p", [128, 2 * NSUP * 64], bf16, kind="ExternalInput")
    dt_ld = dt.float8e4 if lhsd_fp8 else bf16
    LHS_d = nc.dram_tensor("LHS_d", [128, 2 * NSUP * 64], dt_ld, kind="ExternalInput")
    GR_p = nc.dram_tensor("GR_p", [128, N], bf16, kind="ExternalInput")
    GR_d = nc.dram_tensor("GR_d", [128, N], bf16, kind="ExternalInput")
    XB = nc.dram_tensor("XB", [128, E], f32, kind="ExternalInput")      # xbar slice
    XDEC = nc.dram_tensor("XDEC", [128, E], f32, kind="ExternalInput")  # (1-ax)*Xd
    UP = nc.dram_tensor("UP", [128, 128], f32, kind="ExternalInput")    # u_pot
    UD = nc.dram_tensor("UD", [128, 128], f32, kind="ExternalInput")    # u_dep
    VP1 = nc.dram_tensor("VP1", [128, 128], f32, kind="ExternalInput")  # (1-ap)*V
    VP2 = nc.dram_tensor("VP2", [128, 128], f32, kind="ExternalInput")  # (1-ad)*V

    W_new = nc.dram_tensor("W_new", [NSUP, 128, N], f32, kind="ExternalOutput")
    XB_new = nc.dram_tensor("XB_new", [128, E], f32, kind="ExternalOutput")
    UP_new = nc.dram_tensor("UP_new", [128, 128], f32, kind="ExternalOutput")
    UD_new = nc.dram_tensor("UD_new", [128, 128], f32, kind="ExternalOutput")

    mul = mybir.AluOpType.mult
    add = mybir.AluOpType.add
    sub = mybir.AluOpType.subtract
    amin = mybir.AluOpType.min
    amax = mybir.AluOpType.max

    with TileContext(nc) as tc:
        with (
            tc.tile_pool(name="const", bufs=1) as cpool,
            tc.tile_pool(name="rhs", bufs=rhs_bufs) as rhs_pool,
            tc.tile_pool(name="wio", bufs=2) as w_pool,
            tc.tile_pool(name="ev", bufs=ev_bufs) as ev_pool,
            # wide mode uses (128, 1024) = 2-bank PSUM tiles; 2 tags x 2 bufs
            # x 2 banks = all 8 banks.
            tc.tile_pool(name="psum",
                         bufs=(2 if ev_mode in ('wide', 'wide2') else psum_bufs),
                         space="PSUM") as psum_pool,
        ):
            # One-time constant loads.  In 'wide2' mode the slices needed by
            # the first super load first as separate small tiles so the first
            # compute chain starts after ~200 KB instead of ~2.2 MB.
            fine = ev_mode == 'wide2'
            if fine:
                lhs_p0 = cpool.tile([128, 128], bf16, tag="lhs_p0")
                nc.scalar.dma_start(lhs_p0[:], LHS_p[:, 0:128])
                lhs_d0 = cpool.tile([128, 128], dt_ld, tag="lhs_d0")
                nc.scalar.dma_start(lhs_d0[:], LHS_d[:, 0:128])
                gr_p0 = cpool.tile([128, 1024], bf16, tag="gr_p0")
                nc.scalar.dma_start(gr_p0[:], GR_p[:, 0:1024])
                gr_d0 = cpool.tile([128, 1024], bf16, tag="gr_d0")
                nc.scalar.dma_start(gr_d0[:], GR_d[:, 0:1024])
            head = False  # head reorder measured neutral; stream-paced
            h_tiles = {}
            if head:
                # Interleave super-0's loads with the constants in dependency
                # order, so the first matmul->evac->gate chain starts after
                # ~1 MB instead of ~2.3 MB of DMA.
                lhs_p = cpool.tile([128, 2 * NSUP * 64], bf16, tag="lhs_p")
                nc.sync.dma_start(lhs_p[:], LHS_p[:])
                h_tiles['rp0'] = rhs_pool.tile([128, N], dt_p, tag="rp0", name="h_rp0")
                nc.sync.dma_start(h_tiles['rp0'][:], DA_p[0, 0])
                h_tiles['rp1'] = rhs_pool.tile([128, N], dt_p, tag="rp1", name="h_rp1")
                nc.sync.dma_start(h_tiles['rp1'][:], DA_p[0, 1])
                gr_p = cpool.tile([128, N], bf16, tag="gr_p")
                nc.sync.dma_start(gr_p[:], GR_p[:])
                lhs_d = cpool.tile([128, 2 * NSUP * 64], dt_ld, tag="lhs_d")
                nc.sync.dma_start(lhs_d[:], LHS_d[:])
                h_tiles['rd0'] = rhs_pool.tile([128, N], dt_d, tag="rd0", name="h_rd0")
                nc.sync.dma_start(h_tiles['rd0'][:], DA_d[0, 0])
                h_tiles['rd1'] = rhs_pool.tile([128, N], dt_d, tag="rd1", name="h_rd1")
                nc.sync.dma_start(h_tiles['rd1'][:], DA_d[0, 1])
                gr_d = cpool.tile([128, N], bf16, tag="gr_d")
                nc.sync.dma_start(gr_d[:], GR_d[:])
                h_tiles['wt'] = w_pool.tile([128, N], f32, tag="wt", name="h_wt")
                nc.sync.dma_start(h_tiles['wt'][:], W_in[0])
            else:
                lhs_p = cpool.tile([128, 2 * NSUP * 64], bf16, tag="lhs_p")
                nc.sync.dma_start(lhs_p[:], LHS_p[:])
                lhs_d = cpool.tile([128, 2 * NSUP * 64], dt_ld, tag="lhs_d")
                nc.sync.dma_start(lhs_d[:], LHS_d[:])
                gr_p = cpool.tile([128, N], bf16, tag="gr_p")
                nc.sync.dma_start(gr_p[:], GR_p[:])
                gr_d = cpool.tile([128, N], bf16, tag="gr_d")
                nc.sync.dma_start(gr_d[:], GR_d[:])

            def emit_traces():
                # Trace updates (tiny).
                xb = ev_pool.tile([128, E], f32, tag="xb")
                nc.sync.dma_start(xb[:], XB[:])
                xd = ev_pool.tile([128, E], f32, tag="xd")
                nc.sync.dma_start(xd[:], XDEC[:])
                xn = ev_pool.tile([128, E], f32, tag="xn")
                nc.vector.scalar_tensor_tensor(xn[:], xb[:], ALPHA_X, xd[:], mul, add)
                nc.sync.dma_start(XB_new[:], xn[:])

                up = ev_pool.tile([128, 128], f32, tag="up")
                nc.sync.dma_start(up[:], UP[:])
                v1 = ev_pool.tile([128, 128], f32, tag="v1")
                nc.sync.dma_start(v1[:], VP1[:])
                un = ev_pool.tile([128, 128], f32, tag="un")
                nc.vector.scalar_tensor_tensor(un[:], up[:], ALPHA_P, v1[:], mul, add)
                nc.sync.dma_start(UP_new[:], un[:])

                ud = ev_pool.tile([128, 128], f32, tag="ud")
                nc.sync.dma_start(ud[:], UD[:])
                v2 = ev_pool.tile([128, 128], f32, tag="v2")
                nc.sync.dma_start(v2[:], VP2[:])
                un2 = ev_pool.tile([128, 128], f32, tag="un2")
                nc.vector.scalar_tensor_tensor(un2[:], ud[:], ALPHA_D, v2[:], mul, add)
                nc.sync.dma_start(UD_new[:], un2[:])

            for r in range(repeat):
                for s in range(NSUP):
                    if head and r == 0 and s == 0:
                        rp0, rp1 = h_tiles['rp0'], h_tiles['rp1']
                        rd0, rd1 = h_tiles['rd0'], h_tiles['rd1']
                        wt = h_tiles['wt']
                    elif ev_mode == 'wide2' and s == 0:
                        rp0 = rp1 = rd0 = rd1 = wt = None  # fine-grained below
                    elif blob:
                        bt = rhs_pool.tile([128, 8 * N], dt.uint8, tag="bt")
                        nc.sync.dma_start(bt[:], BLOB[s])
                        rp0 = bt[:, 0 * N:1 * N].bitcast(dt_p)
                        rp1 = bt[:, 1 * N:2 * N].bitcast(dt_p)
                        rd0 = bt[:, 2 * N:3 * N].bitcast(dt_d)
                        rd1 = bt[:, 3 * N:4 * N].bitcast(dt_d)
                        wt = bt[:, 4 * N:8 * N].bitcast(f32)
                    else:
                        rp0 = rhs_pool.tile([128, N], dt_p, tag="rp0")
                        rp1 = rhs_pool.tile([128, N], dt_p, tag="rp1")
                        rd0 = rhs_pool.tile([128, N], dt_d, tag="rd0")
                        rd1 = rhs_pool.tile([128, N], dt_d, tag="rd1")
                        wt = w_pool.tile([128, N], f32, tag="wt")
                        if dma_split:
                            h = N // 2
                            for t_, src in ((rp0, DA_p[s, 0]), (rp1, DA_p[s, 1]),
                                            (rd0, DA_d[s, 0]), (rd1, DA_d[s, 1]),
                                            (wt, W_in[s])):
                                nc.sync.dma_start(t_[:, 0:h], src[:, 0:h])
                                nc.sync.dma_start(t_[:, h:N], src[:, h:N])
                        else:
                            nc.sync.dma_start(rp0[:], DA_p[s, 0])
                            nc.sync.dma_start(rp1[:], DA_p[s, 1])
                            rd_eng = nc.gpsimd if rd_gp else (nc.scalar if rd_act else nc.sync)
                            rd_eng.dma_start(rd0[:], DA_d[s, 0])
                            rd_eng.dma_start(rd1[:], DA_d[s, 1])
                            (nc.gpsimd if w_gp else (nc.scalar if w_act else nc.sync)).dma_start(wt[:], W_in[s])
                    wo = w_pool.tile([128, N], f32, tag="wo")

                    j0, j1 = 2 * s, 2 * s + 1
                    if _skip == 'dmaonly':
                        nc.vector.tensor_copy(wo[:, 0:512], wt[:, 0:512])
                        nc.sync.dma_start(W_new[s], wo[:])
                        continue
                    if ev_mode == 'wide2' and s == 0:
                        # First super at oc granularity with per-chunk DMAs
                        # into separate tiles: the first matmul/DVE chain
                        # starts as soon as its ~200 KB lands.
                        for oc in range(OC):
                            osl = bass.ts(oc, 512)
                            fr0 = rhs_pool.tile([128, 512], dt_p, tag="f0")
                            fr1 = rhs_pool.tile([128, 512], dt_p, tag="f1")
                            fr2 = rhs_pool.tile([128, 512], dt_d, tag="f2")
                            fr3 = rhs_pool.tile([128, 512], dt_d, tag="f3")
                            nc.scalar.dma_start(fr0[:], DA_p[0, 0][:, osl])
                            nc.scalar.dma_start(fr1[:], DA_p[0, 1][:, osl])
                            nc.scalar.dma_start(fr2[:], DA_d[0, 0][:, osl])
                            nc.scalar.dma_start(fr3[:], DA_d[0, 1][:, osl])
                            fw = rhs_pool.tile([128, 512], f32, tag="fw")
                            nc.scalar.dma_start(fw[:], W_in[0][:, osl])
                            pp = psum_pool.tile([128, 512], f32, tag="pp")
                            nc.tensor.matmul(pp[0:64, :], lhs_p0[:, 0:64],
                                             fr0[:], start=True, stop=True)
                            nc.tensor.matmul(pp[64:128, :], lhs_p0[:, 64:128],
                                             fr1[:], start=True, stop=True)
                            pd = psum_pool.tile([128, 512], f32, tag="pd")
                            nc.tensor.matmul(pd[0:64, :], lhs_d0[:, 0:64],
                                             fr2[:], start=True, stop=True)
                            nc.tensor.matmul(pd[64:128, :], lhs_d0[:, 64:128],
                                             fr3[:], start=True, stop=True)
                            sp = ev_pool.tile([128, 512], bf16, tag="fsp")
                            nc.scalar.copy(sp[:], pp[:])
                            sd = ev_pool.tile([128, 512], bf16, tag="fsd")
                            nc.scalar.copy(sd[:], pd[:])
                            gp_src = gr_p0 if oc < 2 else gr_p
                            gd_src = gr_d0 if oc < 2 else gr_d
                            t1 = ev_pool.tile([128, 512], bf16, tag="ft1")
                            nc.vector.tensor_tensor(t1[:], sp[:], gp_src[:, osl], mul)
                            t2 = ev_pool.tile([128, 512], bf16, tag="ft2")
                            nc.vector.tensor_tensor(t2[:], sd[:], gd_src[:, osl], mul)
                            t3 = ev_pool.tile([128, 512], f32, tag="ft3")
                            nc.vector.tensor_tensor(t3[:], t1[:], t2[:], sub)
                            t4 = ev_pool.tile([128, 512], f32, tag="ft4")
                            nc.vector.tensor_tensor(t4[:], t3[:], fw[:], add)
                            nc.vector.tensor_scalar(wo[:, osl], t4[:], WMAX, 0.0,
                                                    amin, amax)
                        (nc.scalar if out_act else nc.sync).dma_start(W_new[0], wo[:])
                        continue
                    narrow_tail = False  # measured worse: narrow DVE ops cost more than the shorter tail chain saves
                    if ev_mode in ('wide', 'wide2'):
                        # oc-pair granularity: (128, 1024) PSUM tiles (2 banks),
                        # one ScalarE evacuation + 1024-wide DVE ops per pair -
                        # amortizes the per-op fixed cost with identical math.
                        last = (ev_mode == 'wide2' and s == NSUP - 1) or \
                               (ev_mode == 'wide' and s >= NSUP - _tail_split)
                        # split@1024 stores release DMA work at finer grain;
                        # best in sweep: split every super
                        for ocp in range(OC // 2):
                            pp = psum_pool.tile([128, 1024], f32, tag="pp")
                            pd = psum_pool.tile([128, 1024], f32, tag="pd")
                            for w in range(2):
                                oc = 2 * ocp + w
                                osl = bass.ts(oc, 512)
                                wsl = slice(w * 512, (w + 1) * 512)
                                nc.tensor.matmul(pp[0:64, wsl], lhs_p[:, bass.ts(j0, 64)],
                                                 rp0[:, osl], start=True, stop=True)
                                nc.tensor.matmul(pp[64:128, wsl], lhs_p[:, bass.ts(j1, 64)],
                                                 rp1[:, osl], start=True, stop=True)
                                nc.tensor.matmul(pd[0:64, wsl], lhs_d[:, bass.ts(j0, 64)],
                                                 rd0[:, osl], start=True, stop=True)
                                nc.tensor.matmul(pd[64:128, wsl], lhs_d[:, bass.ts(j1, 64)],
                                                 rd1[:, osl], start=True, stop=True)
                            psl = bass.ts(ocp, 1024)
                            sp = ev_pool.tile([128, 1024], bf16, tag="sp")
                            nc.scalar.copy(sp[:], pp[:])
                            sd = ev_pool.tile([128, 1024], bf16, tag="sd")
                            nc.scalar.copy(sd[:], pd[:])
                            t1 = ev_pool.tile([128, 1024], bf16, tag="t1")
                            nc.vector.tensor_tensor(t1[:], sp[:], gr_p[:, psl], mul)
                            t2 = ev_pool.tile([128, 1024], bf16, tag="t2")
                            nc.vector.tensor_tensor(t2[:], sd[:], gr_d[:, psl], mul)
                            t3 = ev_pool.tile([128, 1024], f32, tag="t3")
                            nc.vector.tensor_tensor(t3[:], t1[:], t2[:], sub)
                            t4 = ev_pool.tile([128, 1024], f32, tag="t4")
                            nc.vector.tensor_tensor(t4[:], t3[:], wt[:, psl], add)
                            nc.vector.tensor_scalar(wo[:, psl], t4[:], WMAX, 0.0,
                                                    amin, amax)
                            if last:
                                # split stores so the final chunk's store is
                                # the only thing on the kernel tail
                                (nc.scalar if out_act else nc.sync).dma_start(
                                    W_new[s][:, psl], wo[:, psl])
                        if not last:
                            (nc.scalar if out_act else nc.sync).dma_start(
                                W_new[s], wo[:])
                        continue
                    for oc in range(OC):
                        osl = bass.ts(oc, 512)
                        pp = psum_pool.tile([128, 512], f32, tag="pp")
                        nc.tensor.matmul(pp[0:64, :], lhs_p[:, bass.ts(j0, 64)],
                                         rp0[:, osl], start=True, stop=True)
                        nc.tensor.matmul(pp[64:128, :], lhs_p[:, bass.ts(j1, 64)],
                                         rp1[:, osl], start=True, stop=True)
                        pd = psum_pool.tile([128, 512], f32, tag="pd")
                        nc.tensor.matmul(pd[0:64, :], lhs_d[:, bass.ts(j0, 64)],
                                         rd0[:, osl], start=True, stop=True)
                        nc.tensor.matmul(pd[64:128, :], lhs_d[:, bass.ts(j1, 64)],
                                         rd1[:, osl], start=True, stop=True)

                        if _skip == 'noev':
                            if oc == 0:
                                nc.vector.tensor_copy(wo[:, 0:512], pp[:])
                            continue
                        sp = ev_pool.tile([128, 512], bf16, tag="sp")
                        nc.scalar.copy(sp[:], pp[:])
                        sd = ev_pool.tile([128, 512], bf16, tag="sd")
                        nc.scalar.copy(sd[:], pd[:])

                        t1 = ev_pool.tile([128, 512], bf16, tag="t1")
                        nc.vector.tensor_tensor(t1[:], sp[:], gr_p[:, osl], mul)
                        t2 = ev_pool.tile([128, 512], bf16, tag="t2")
                        nc.vector.tensor_tensor(t2[:], sd[:], gr_d[:, osl], mul)
                        t3 = ev_pool.tile([128, 512], f32, tag="t3")
                        if ev_mode == 'subvec':
                            nc.vector.tensor_tensor(t3[:], t1[:], t2[:], sub)
                        else:
                            nc.gpsimd.tensor_tensor(t3[:], t1[:], t2[:], sub)
                        t4 = ev_pool.tile([128, 512], f32, tag="t4")
                        if ev_mode == 'v4gp':
                            nc.gpsimd.tensor_tensor(t4[:], t3[:], wt[:, osl], add)
                        else:
                            nc.vector.tensor_tensor(t4[:], t3[:], wt[:, osl], add)
                        if ev_mode == 'clip_gp':
                            nc.gpsimd.tensor_scalar(wo[:, osl], t4[:], WMAX, 0.0,
                                                    amin, amax)
                        else:
                            nc.vector.tensor_scalar(wo[:, osl], t4[:], WMAX, 0.0,
                                                    amin, amax)
                    (nc.scalar if out_act else nc.sync).dma_start(W_new[s], wo[:])
                emit_traces()

    nc.finalize()
    return nc


def prepare_inputs(Xd, Xpost, Vpost, xbar_pre, u_pot, u_dep, W, A_p, A_d, dmap,
                   fp8_pot=False, fp8_dep=False, blob=False, lhsd_fp8=False):
    """Host-side shard + repack.  Returns list of per-core input dicts.

    With fp8_*: the folded dmap*A tensor is stored as float8_e4m3 scaled up
    by FP8_SCALE (A ~ 1e-3 underflows e4m3 subnormals); the matching
    replicated gate tensor is scaled down by 1/FP8_SCALE to compensate
    (exact power-of-two, no precision loss).
    """
    gp = (Xpost * np.maximum(u_pot, 0.0)).astype(F32)        # (B, N)
    gd = np.maximum(u_dep, 0.0).astype(F32)
    FP8 = ml_dtypes.float8_e4m3
    sp = 1.0 / FP8_SCALE if fp8_pot else 1.0
    sd = 1.0 / FP8_SCALE if fp8_dep else 1.0
    grep_p = np.tile(np.repeat(gp * sp, 8, axis=0), (2, 1)).astype(BF16)  # (128, N)
    grep_d = np.tile(np.repeat(gd * sd, 8, axis=0), (2, 1)).astype(BF16)

    if fp8_pot:
        DA_p_full = (dmap * (A_p * FP8_SCALE)[None]).astype(FP8)
    else:
        DA_p_full = (dmap * A_p[None]).astype(BF16)          # (D, N, N)
    if fp8_dep:
        DA_d_full = (dmap * (A_d * FP8_SCALE)[None]).astype(FP8)
    else:
        DA_d_full = (dmap * A_d[None]).astype(BF16)

    xb_flat = xbar_pre.reshape(D * B, N)
    xdec_flat = (np.float32(1.0 - ALPHA_X) * Xd).reshape(D * B, N).astype(F32)
    up_r = u_pot.reshape(128, 128).astype(F32)
    ud_r = u_dep.reshape(128, 128).astype(F32)
    vp1 = (np.float32(1.0 - ALPHA_P) * Vpost).reshape(128, 128).astype(F32)
    vp2 = (np.float32(1.0 - ALPHA_D) * Vpost).reshape(128, 128).astype(F32)

    def pack_lhs(src, dtype):
        # src: (D, B, N) -> per-core (128, 2*NSUP*64) block-diagonal lhsT.
        out = []
        for ci in range(NCORES):
            sl = slice(ci * E, (ci + 1) * E)
            xs = src[:, :, sl].reshape(D, B, 2 * NSUP, 8)    # d, b, j, el
            blk = np.zeros((2 * NSUP, 8, D, B, 8), dtype=F32)  # j, el_k, d, b, el_m
            for el in range(8):
                blk[:, el, :, :, el] = xs[:, :, :, el].transpose(2, 0, 1)
            # k = el*16 + d (el outer), col = j*64 + b*8 + el_m
            lhs = blk.reshape(2 * NSUP, 128, 64).transpose(1, 0, 2).reshape(128, -1)
            out.append(np.ascontiguousarray(lhs).astype(dtype))
        return out

    lhs_p_cores = pack_lhs(xbar_pre, BF16)
    # Xd is binary 0/1 - exact in fp8, halves the lhsT bytes.
    lhs_d_cores = pack_lhs(Xd, FP8 if lhsd_fp8 else BF16)

    in_maps = []
    for ci in range(NCORES):
        sl = slice(ci * E, (ci + 1) * E)

        def pack_da(full):
            # (D, E, N) -> (NSUP, 2, 128, N), k = el*16 + d
            x = full[:, sl].reshape(D, NSUP, 2, 8, N).transpose(1, 2, 3, 0, 4)
            return np.ascontiguousarray(x.reshape(NSUP, 2, 128, N))

        w_c = W[:, sl].reshape(B, NSUP, 2, 8, N).transpose(1, 2, 0, 3, 4)
        w_c = np.ascontiguousarray(w_c.reshape(NSUP, 128, N), dtype=F32)

        if blob:
            pda_p = pack_da(DA_p_full)  # (NSUP, 2, 128, N) fp8
            pda_d = pack_da(DA_d_full)
            bl = np.empty((NSUP, 128, 8 * N), np.uint8)
            bl[:, :, 0 * N:1 * N] = pda_p[:, 0].view(np.uint8)
            bl[:, :, 1 * N:2 * N] = pda_p[:, 1].view(np.uint8)
            bl[:, :, 2 * N:3 * N] = pda_d[:, 0].view(np.uint8)
            bl[:, :, 3 * N:4 * N] = pda_d[:, 1].view(np.uint8)
            bl[:, :, 4 * N:8 * N] = w_c.view(np.uint8)
            io = dict(BLOB=bl)
        else:
            io = dict(DA_p=pack_da(DA_p_full), DA_d=pack_da(DA_d_full), W_in=w_c)

        in_maps.append(dict(
            **io,
            LHS_p=lhs_p_cores[ci], LHS_d=lhs_d_cores[ci],
            GR_p=grep_p, GR_d=grep_d,
            XB=np.ascontiguousarray(xb_flat[:, sl], dtype=F32),
            XDEC=np.ascontiguousarray(xdec_flat[:, sl]),
            UP=up_r, UD=ud_r, VP1=vp1, VP2=vp2,
        ))
    return in_maps


def assemble_outputs(results, W):
    W_new = np.empty((B, N, N), dtype=F32)
    xbar_new = np.empty((D * B, N), dtype=F32)
    for ci in range(NCORES):
        sl = slice(ci * E, (ci + 1) * E)
        wn = results[ci]["W_new"].reshape(NSUP, 2, B, 8, N).transpose(2, 0, 1, 3, 4)
        W_new[:, sl] = wn.reshape(B, E, N)
        xbar_new[:, sl] = results[ci]["XB_new"]
    u_pot_new = results[0]["UP_new"].reshape(B, N)
    u_dep_new = results[0]["UD_new"].reshape(B, N)
    out = np.asarray(W, dtype=F32)
    return out, W_new, xbar_new.reshape(D, B, N), u_pot_new, u_dep_new


# Production configuration: fp8 folded dmap*A tensors (exact 0/1 dmap,
# power-of-two compensation in the gate tensors), all-DVE elementwise,
# W_new stores issued from the ACT HWDGE ring.
KERNEL_CFG = dict(fp8_pot=True, fp8_dep=True, ev_mode="wide", out_act=True,
                  lhsd_fp8=True)

_NC_CACHE: dict = {}


def _get_nc():
    if "nc" not in _NC_CACHE:
        _NC_CACHE["nc"] = build_kernel(**KERNEL_CFG)
    return _NC_CACHE["nc"]


def kernel(Xd, Xpost, Vpost, xbar_pre, u_pot, u_dep, W, A_p, A_d, dmap):
    args = [np.asarray(a, dtype=F32) for a in
            (Xd, Xpost, Vpost, xbar_pre, u_pot, u_dep, W, A_p, A_d, dmap)]
    in_maps = prepare_inputs(*args, fp8_pot=KERNEL_CFG["fp8_pot"],
                             fp8_dep=KERNEL_CFG["fp8_dep"],
                             lhsd_fp8=KERNEL_CFG["lhsd_fp8"])
    nc = _get_nc()
    # The axon-tunneled device occasionally reports a transient
    # NRT_EXEC_UNIT_UNRECOVERABLE that clears after the remote worker
    # restarts; retry a couple of times before giving up.
    last_exc = None
    for attempt in range(3):
        try:
            res = run_bass_kernel_spmd(nc, in_maps, core_ids=list(range(NCORES)))
            return assemble_outputs(res.results, args[6])
        except Exception as exc:  # noqa: BLE001
            last_exc = exc
            if attempt < 2:
                import time
                time.sleep(45)
    raise last_exc


# revision 46
# speedup vs baseline: 1.0046x; 1.0046x over previous
"""Trainium2 Bass kernel for the Clopath plasticity rule (nn_Clopath).

Math (reference):
    dW_pot[b,e,o] = sum_d xbar[d,b,e] * dmap[d,e,o] * A_p[e,o] * gp[b,o]
    dW_dep[b,e,o] = sum_d Xd[d,b,e]   * dmap[d,e,o] * A_d[e,o] * gd[b,o]
        gp = Xpost * relu(u_pot),  gd = relu(u_dep)
    W_new = clip(W + dW_pot - dW_dep, 0, 2)
    out = W (pre-update);  plus three exponential trace updates.

Strategy (8 NeuronCores, shard pre-synaptic dim e; DMA-bound at ~161 us/core
per the instruction cost model, vs a ~149 us pure-bytes floor):
  * Host folds A into dmap:  DA_p = dmap*A_p, DA_d = dmap*A_d, stored as
    float8_e4m3 scaled by 256 (A ~ 1e-3 underflows e4m3; the 1/256
    compensation goes into the bf16 gate tensors - exact power of two).
    dmap is 0/1 so fp8 quantizes only A (~6% worst-case on a term that is
    ~1e-4 of W; measured absmax error 1.2e-4 relative on W_new).
  * The d-contraction runs on TensorE as block-diagonal packed matmuls:
    contraction K = 8 e's x 16 d = 128, out M = 64 = (b x e_local), N = 512
    o-chunk.  lhsT carries xbar (bf16) / Xd (fp8, exact 0/1) values on the
    block diagonal (built on host); rhs is a (128, 2048) repack of DA rows.
    Two matmuls (two e-halves) fill a (128, 512) PSUM tile at partition
    offsets 0/64.
  * Gating by gp/gd uses partition-replicated (128, N) bf16 tiles; ScalarE
    evacuates PSUM to bf16, VectorE does the gate multiplies (bf16 2x
    mode), pot-dep subtract, fp32 W add, and a fused min/max clip; W_new
    stores issue from the ACT HWDGE ring to overlap with SP-ring loads.
  * Trace updates run on-device as single fused scalar_tensor_tensor ops.
  * All elementwise/PE work hides under the DMA stream (W fp32 in+out
    33.6 MB/core + DA fp8 16.8 MB/core dominate).
"""

import numpy as np
import ml_dtypes

import concourse.bass as bass
import concourse.bacc as bacc
import concourse.mybir as mybir
from concourse.tile import TileContext
from concourse.bass_utils import run_bass_kernel_spmd

BF16 = ml_dtypes.bfloat16
F32 = np.float32

D, B, N = 16, 8, 2048
NCORES = 8
E = N // NCORES          # 256 pre-synaptic neurons per core
NSUP = E // 16           # 16 supers (16 e's each) per core
ALPHA_X, ALPHA_P, ALPHA_D = 0.95, 0.9, 0.8
WMAX = 2.0
OC = N // 512            # 4 o-chunks of 512


FP8_SCALE = 256.0


def build_kernel(repeat: int = 1, fp8_pot: bool = False, fp8_dep: bool = False,
                 rhs_bufs: int = 2, ev_bufs: int = 3, blob: bool = False,
                 ev_mode: str = 'default', dma_split: bool = False,
                 out_act: bool = False, w_act: bool = False,
                 psum_bufs: int = 3, lhsd_fp8: bool = False,
                 rd_act: bool = False, rd_gp: bool = False,
                 w_gp: bool = False, _tail_split: int = 16,
                 gr_pe: bool = False, _skip: str = '') -> bass.Bass:
    if blob:
        assert fp8_pot and fp8_dep, "blob layout assumes fp8 DA tensors" 
    # Bacc (not plain Bass): its finalize() runs move_matmul_waits_to_
    # ldweights + generate_event_semaphores, which split multi-sem waits to
    # satisfy the 1-wait-per-instruction TRN2 codegen constraint.
    nc = bacc.Bacc()
    dt = mybir.dt
    f32, bf16 = dt.float32, dt.bfloat16
    dt_p = dt.float8e4 if fp8_pot else bf16
    dt_d = dt.float8e4 if fp8_dep else bf16

    # Per-core inputs (host pre-packed into DMA-friendly layouts).
    if blob:
        # One byte-interleaved tensor per super: per partition row =
        # [DA_p h0 | DA_p h1 | DA_d h0 | DA_d h1 (fp8, N bytes each) | W (f32, 4N bytes)]
        BLOB = nc.dram_tensor("BLOB", [NSUP, 128, 8 * N], dt.uint8,
                              kind="ExternalInput")
    else:
        DA_p = nc.dram_tensor("DA_p", [NSUP, 2, 128, N], dt_p, kind="ExternalInput")
        DA_d = nc.dram_tensor("DA_d", [NSUP, 2, 128, N], dt_d, kind="ExternalInput")
        W_in = nc.dram_tensor("W_in", [NSUP, 128, N], f32, kind="ExternalInput")
    LHS_p = nc.dram_tensor("LHS_p", [128, 2 * NSUP * 64], bf16, kind="ExternalInput")
    dt_ld = dt.float8e4 if lhsd_fp8 else bf16
    LHS_d = nc.dram_tensor("LHS_d", [128, 2 * NSUP * 64], dt_ld, kind="ExternalInput")
    if gr_pe:
        GRC_p = nc.dram_tensor("GRC_p", [8, N], bf16, kind="ExternalInput")
        GRC_d = nc.dram_tensor("GRC_d", [8, N], bf16, kind="ExternalInput")
        REP = nc.dram_tensor("REP", [8, 128], bf16, kind="ExternalInput")
    else:
        GR_p = nc.dram_tensor("GR_p", [128, N], bf16, kind="ExternalInput")
        GR_d = nc.dram_tensor("GR_d", [128, N], bf16, kind="ExternalInput")
    XB = nc.dram_tensor("XB", [128, E], f32, kind="ExternalInput")      # xbar slice
    XDEC = nc.dram_tensor("XDEC", [128, E], f32, kind="ExternalInput")  # (1-ax)*Xd
    UP = nc.dram_tensor("UP", [128, 128], f32, kind="ExternalInput")    # u_pot
    UD = nc.dram_tensor("UD", [128, 128], f32, kind="ExternalInput")    # u_dep
    VP1 = nc.dram_tensor("VP1", [128, 128], f32, kind="ExternalInput")  # (1-ap)*V
    VP2 = nc.dram_tensor("VP2", [128, 128], f32, kind="ExternalInput")  # (1-ad)*V

    W_new = nc.dram_tensor("W_new", [NSUP, 128, N], f32, kind="ExternalOutput")
    XB_new = nc.dram_tensor("XB_new", [128, E], f32, kind="ExternalOutput")
    UP_new = nc.dram_tensor("UP_new", [128, 128], f32, kind="ExternalOutput")
    UD_new = nc.dram_tensor("UD_new", [128, 128], f32, kind="ExternalOutput")

    mul = mybir.AluOpType.mult
    add = mybir.AluOpType.add
    sub = mybir.AluOpType.subtract
    amin = mybir.AluOpType.min
    amax = mybir.AluOpType.max

    with TileContext(nc) as tc:
        with (
            tc.tile_pool(name="const", bufs=1) as cpool,
            tc.tile_pool(name="rhs", bufs=rhs_bufs) as rhs_pool,
            tc.tile_pool(name="wio", bufs=2) as w_pool,
            tc.tile_pool(name="ev", bufs=ev_bufs) as ev_pool,
            # wide mode uses (128, 1024) = 2-bank PSUM tiles; 2 tags x 2 bufs
            # x 2 banks = all 8 banks.
            tc.tile_pool(name="psum",
                         bufs=(2 if ev_mode in ('wide', 'wide2') else psum_bufs),
                         space="PSUM") as psum_pool,
        ):
            # One-time constant loads.  In 'wide2' mode the slices needed by
            # the first super load first as separate small tiles so the first
            # compute chain starts after ~200 KB instead of ~2.2 MB.
            fine = ev_mode == 'wide2'
            if fine:
                lhs_p0 = cpool.tile([128, 128], bf16, tag="lhs_p0")
                nc.scalar.dma_start(lhs_p0[:], LHS_p[:, 0:128])
                lhs_d0 = cpool.tile([128, 128], dt_ld, tag="lhs_d0")
                nc.scalar.dma_start(lhs_d0[:], LHS_d[:, 0:128])
                gr_p0 = cpool.tile([128, 1024], bf16, tag="gr_p0")
                nc.scalar.dma_start(gr_p0[:], GR_p[:, 0:1024])
                gr_d0 = cpool.tile([128, 1024], bf16, tag="gr_d0")
                nc.scalar.dma_start(gr_d0[:], GR_d[:, 0:1024])
            head = False  # head reorder measured neutral; stream-paced
            h_tiles = {}
            if head:
                # Interleave super-0's loads with the constants in dependency
                # order, so the first matmul->evac->gate chain starts after
                # ~1 MB instead of ~2.3 MB of DMA.
                lhs_p = cpool.tile([128, 2 * NSUP * 64], bf16, tag="lhs_p")
                nc.sync.dma_start(lhs_p[:], LHS_p[:])
                h_tiles['rp0'] = rhs_pool.tile([128, N], dt_p, tag="rp0", name="h_rp0")
                nc.sync.dma_start(h_tiles['rp0'][:], DA_p[0, 0])
                h_tiles['rp1'] = rhs_pool.tile([128, N], dt_p, tag="rp1", name="h_rp1")
                nc.sync.dma_start(h_tiles['rp1'][:], DA_p[0, 1])
                gr_p = cpool.tile([128, N], bf16, tag="gr_p")
                nc.sync.dma_start(gr_p[:], GR_p[:])
                lhs_d = cpool.tile([128, 2 * NSUP * 64], dt_ld, tag="lhs_d")
                nc.sync.dma_start(lhs_d[:], LHS_d[:])
                h_tiles['rd0'] = rhs_pool.tile([128, N], dt_d, tag="rd0", name="h_rd0")
                nc.sync.dma_start(h_tiles['rd0'][:], DA_d[0, 0])
                h_tiles['rd1'] = rhs_pool.tile([128, N], dt_d, tag="rd1", name="h_rd1")
                nc.sync.dma_start(h_tiles['rd1'][:], DA_d[0, 1])
                gr_d = cpool.tile([128, N], bf16, tag="gr_d")
                nc.sync.dma_start(gr_d[:], GR_d[:])
                h_tiles['wt'] = w_pool.tile([128, N], f32, tag="wt", name="h_wt")
                nc.sync.dma_start(h_tiles['wt'][:], W_in[0])
            else:
                lhs_p = cpool.tile([128, 2 * NSUP * 64], bf16, tag="lhs_p")
                nc.sync.dma_start(lhs_p[:], LHS_p[:])
                lhs_d = cpool.tile([128, 2 * NSUP * 64], dt_ld, tag="lhs_d")
                nc.sync.dma_start(lhs_d[:], LHS_d[:])
                gr_p = cpool.tile([128, N], bf16, tag="gr_p")
                gr_d = cpool.tile([128, N], bf16, tag="gr_d")
                if gr_pe:
                    # Build the partition-replicated gate tensors on-chip:
                    # PE multiplies the compact (8, N) rows by a 0/1 selector
                    # (exact, x1.0 accumulate) - saves ~1 MB of HBM traffic.
                    grc_p = cpool.tile([8, N], bf16, tag="grc_p")
                    nc.scalar.dma_start(grc_p[:], GRC_p[:])
                    grc_d = cpool.tile([8, N], bf16, tag="grc_d")
                    nc.scalar.dma_start(grc_d[:], GRC_d[:])
                    rept = cpool.tile([8, 128], bf16, tag="rept")
                    nc.scalar.dma_start(rept[:], REP[:])
                    for gsrc, gdst, ptag in ((grc_p, gr_p, "pp"), (grc_d, gr_d, "pd")):
                        for h in range(N // 1024):
                            rt = psum_pool.tile([128, 1024], f32, tag=ptag)
                            for q in range(2):
                                c0 = h * 1024 + q * 512
                                nc.tensor.matmul(rt[:, q * 512:(q + 1) * 512],
                                                 rept[:], gsrc[:, c0:c0 + 512],
                                                 start=True, stop=True)
                            nc.scalar.copy(gdst[:, h * 1024:(h + 1) * 1024], rt[:])
                else:
                    # gate tensors via the ACT ring: it is store-only otherwise,
                    # so its sequencer is idle at t=0 and the transfers overlap
                    # the SP ring's constant loads.
                    nc.scalar.dma_start(gr_p[:], GR_p[:])
                    nc.scalar.dma_start(gr_d[:], GR_d[:])

            def emit_traces():
                # Trace updates (tiny).
                xb = ev_pool.tile([128, E], f32, tag="xb")
                nc.sync.dma_start(xb[:], XB[:])
                xd = ev_pool.tile([128, E], f32, tag="xd")
                nc.sync.dma_start(xd[:], XDEC[:])
                xn = ev_pool.tile([128, E], f32, tag="xn")
                nc.vector.scalar_tensor_tensor(xn[:], xb[:], ALPHA_X, xd[:], mul, add)
                nc.sync.dma_start(XB_new[:], xn[:])

                up = ev_pool.tile([128, 128], f32, tag="up")
                nc.sync.dma_start(up[:], UP[:])
                v1 = ev_pool.tile([128, 128], f32, tag="v1")
                nc.sync.dma_start(v1[:], VP1[:])
                un = ev_pool.tile([128, 128], f32, tag="un")
                nc.vector.scalar_tensor_tensor(un[:], up[:], ALPHA_P, v1[:], mul, add)
                nc.sync.dma_start(UP_new[:], un[:])

                ud = ev_pool.tile([128, 128], f32, tag="ud")
                nc.sync.dma_start(ud[:], UD[:])
                v2 = ev_pool.tile([128, 128], f32, tag="v2")
                nc.sync.dma_start(v2[:], VP2[:])
                un2 = ev_pool.tile([128, 128], f32, tag="un2")
                nc.vector.scalar_tensor_tensor(un2[:], ud[:], ALPHA_D, v2[:], mul, add)
                nc.sync.dma_start(UD_new[:], un2[:])

            for r in range(repeat):
                for s in range(NSUP):
                    if head and r == 0 and s == 0:
                        rp0, rp1 = h_tiles['rp0'], h_tiles['rp1']
                        rd0, rd1 = h_tiles['rd0'], h_tiles['rd1']
                        wt = h_tiles['wt']
                    elif ev_mode == 'wide2' and s == 0:
                        rp0 = rp1 = rd0 = rd1 = wt = None  # fine-grained below
                    elif blob:
                        bt = rhs_pool.tile([128, 8 * N], dt.uint8, tag="bt")
                        nc.sync.dma_start(bt[:], BLOB[s])
                        rp0 = bt[:, 0 * N:1 * N].bitcast(dt_p)
                        rp1 = bt[:, 1 * N:2 * N].bitcast(dt_p)
                        rd0 = bt[:, 2 * N:3 * N].bitcast(dt_d)
                        rd1 = bt[:, 3 * N:4 * N].bitcast(dt_d)
                        wt = bt[:, 4 * N:8 * N].bitcast(f32)
                    else:
                        rp0 = rhs_pool.tile([128, N], dt_p, tag="rp0")
                        rp1 = rhs_pool.tile([128, N], dt_p, tag="rp1")
                        rd0 = rhs_pool.tile([128, N], dt_d, tag="rd0")
                        rd1 = rhs_pool.tile([128, N], dt_d, tag="rd1")
                        wt = w_pool.tile([128, N], f32, tag="wt")
                        if dma_split:
                            h = N // 2
                            for t_, src in ((rp0, DA_p[s, 0]), (rp1, DA_p[s, 1]),
                                            (rd0, DA_d[s, 0]), (rd1, DA_d[s, 1]),
                                            (wt, W_in[s])):
                                nc.sync.dma_start(t_[:, 0:h], src[:, 0:h])
                                nc.sync.dma_start(t_[:, h:N], src[:, h:N])
                        else:
                            nc.sync.dma_start(rp0[:], DA_p[s, 0])
                            nc.sync.dma_start(rp1[:], DA_p[s, 1])
                            rd_eng = nc.gpsimd if rd_gp else (nc.scalar if rd_act else nc.sync)
                            rd_eng.dma_start(rd0[:], DA_d[s, 0])
                            rd_eng.dma_start(rd1[:], DA_d[s, 1])
                            (nc.gpsimd if w_gp else (nc.scalar if w_act else nc.sync)).dma_start(wt[:], W_in[s])
                    wo = w_pool.tile([128, N], f32, tag="wo")

                    j0, j1 = 2 * s, 2 * s + 1
                    if _skip == 'dmaonly':
                        nc.vector.tensor_copy(wo[:, 0:512], wt[:, 0:512])
                        nc.sync.dma_start(W_new[s], wo[:])
                        continue
                    if ev_mode == 'wide2' and s == 0:
                        # First super at oc granularity with per-chunk DMAs
                        # into separate tiles: the first matmul/DVE chain
                        # starts as soon as its ~200 KB lands.
                        for oc in range(OC):
                            osl = bass.ts(oc, 512)
                            fr0 = rhs_pool.tile([128, 512], dt_p, tag="f0")
                            fr1 = rhs_pool.tile([128, 512], dt_p, tag="f1")
                            fr2 = rhs_pool.tile([128, 512], dt_d, tag="f2")
                            fr3 = rhs_pool.tile([128, 512], dt_d, tag="f3")
                            nc.scalar.dma_start(fr0[:], DA_p[0, 0][:, osl])
                            nc.scalar.dma_start(fr1[:], DA_p[0, 1][:, osl])
                            nc.scalar.dma_start(fr2[:], DA_d[0, 0][:, osl])
                            nc.scalar.dma_start(fr3[:], DA_d[0, 1][:, osl])
                            fw = rhs_pool.tile([128, 512], f32, tag="fw")
                            nc.scalar.dma_start(fw[:], W_in[0][:, osl])
                            pp = psum_pool.tile([128, 512], f32, tag="pp")
                            nc.tensor.matmul(pp[0:64, :], lhs_p0[:, 0:64],
                                             fr0[:], start=True, stop=True)
                            nc.tensor.matmul(pp[64:128, :], lhs_p0[:, 64:128],
                                             fr1[:], start=True, stop=True)
                            pd = psum_pool.tile([128, 512], f32, tag="pd")
                            nc.tensor.matmul(pd[0:64, :], lhs_d0[:, 0:64],
                                             fr2[:], start=True, stop=True)
                            nc.tensor.matmul(pd[64:128, :], lhs_d0[:, 64:128],
                                             fr3[:], start=True, stop=True)
                            sp = ev_pool.tile([128, 512], bf16, tag="fsp")
                            nc.scalar.copy(sp[:], pp[:])
                            sd = ev_pool.tile([128, 512], bf16, tag="fsd")
                            nc.scalar.copy(sd[:], pd[:])
                            gp_src = gr_p0 if oc < 2 else gr_p
                            gd_src = gr_d0 if oc < 2 else gr_d
                            t1 = ev_pool.tile([128, 512], bf16, tag="ft1")
                            nc.vector.tensor_tensor(t1[:], sp[:], gp_src[:, osl], mul)
                            t2 = ev_pool.tile([128, 512], bf16, tag="ft2")
                            nc.vector.tensor_tensor(t2[:], sd[:], gd_src[:, osl], mul)
                            t3 = ev_pool.tile([128, 512], f32, tag="ft3")
                            nc.vector.tensor_tensor(t3[:], t1[:], t2[:], sub)
                            t4 = ev_pool.tile([128, 512], f32, tag="ft4")
                            nc.vector.tensor_tensor(t4[:], t3[:], fw[:], add)
                            nc.vector.tensor_scalar(wo[:, osl], t4[:], WMAX, 0.0,
                                                    amin, amax)
                        (nc.scalar if out_act else nc.sync).dma_start(W_new[0], wo[:])
                        continue
                    narrow_tail = False  # measured worse: narrow DVE ops cost more than the shorter tail chain saves
                    if ev_mode in ('wide', 'wide2'):
                        # oc-pair granularity: (128, 1024) PSUM tiles (2 banks),
                        # one ScalarE evacuation + 1024-wide DVE ops per pair -
                        # amortizes the per-op fixed cost with identical math.
                        last = (ev_mode == 'wide2' and s == NSUP - 1) or \
                               (ev_mode == 'wide' and s >= NSUP - _tail_split)
                        # split@1024 stores release DMA work at finer grain;
                        # best in sweep: split every super
                        for ocp in range(OC // 2):
                            pp = psum_pool.tile([128, 1024], f32, tag="pp")
                            pd = psum_pool.tile([128, 1024], f32, tag="pd")
                            for w in range(2):
                                oc = 2 * ocp + w
                                osl = bass.ts(oc, 512)
                                wsl = slice(w * 512, (w + 1) * 512)
                                nc.tensor.matmul(pp[0:64, wsl], lhs_p[:, bass.ts(j0, 64)],
                                                 rp0[:, osl], start=True, stop=True)
                                nc.tensor.matmul(pp[64:128, wsl], lhs_p[:, bass.ts(j1, 64)],
                                                 rp1[:, osl], start=True, stop=True)
                                nc.tensor.matmul(pd[0:64, wsl], lhs_d[:, bass.ts(j0, 64)],
                                                 rd0[:, osl], start=True, stop=True)
                                nc.tensor.matmul(pd[64:128, wsl], lhs_d[:, bass.ts(j1, 64)],
                                                 rd1[:, osl], start=True, stop=True)
                            psl = bass.ts(ocp, 1024)
                            sp = ev_pool.tile([128, 1024], bf16, tag="sp")
                            nc.scalar.copy(sp[:], pp[:])
                            sd = ev_pool.tile([128, 1024], bf16, tag="sd")
                            nc.scalar.copy(sd[:], pd[:])
                            t1 = ev_pool.tile([128, 1024], bf16, tag="t1")
                            nc.vector.tensor_tensor(t1[:], sp[:], gr_p[:, psl], mul)
                            t2 = ev_pool.tile([128, 1024], bf16, tag="t2")
                            nc.vector.tensor_tensor(t2[:], sd[:], gr_d[:, psl], mul)
                            t3 = ev_pool.tile([128, 1024], f32, tag="t3")
                            nc.vector.tensor_tensor(t3[:], t1[:], t2[:], sub)
                            t4 = ev_pool.tile([128, 1024], f32, tag="t4")
                            nc.vector.tensor_tensor(t4[:], t3[:], wt[:, psl], add)
                            nc.vector.tensor_scalar(wo[:, psl], t4[:], WMAX, 0.0,
                                                    amin, amax)
                            if last:
                                # split stores so the final chunk's store is
                                # the only thing on the kernel tail
                                (nc.scalar if out_act else nc.sync).dma_start(
                                    W_new[s][:, psl], wo[:, psl])
                        if not last:
                            (nc.scalar if out_act else nc.sync).dma_start(
                                W_new[s], wo[:])
                        continue
                    for oc in range(OC):
                        osl = bass.ts(oc, 512)
                        pp = psum_pool.tile([128, 512], f32, tag="pp")
                        nc.tensor.matmul(pp[0:64, :], lhs_p[:, bass.ts(j0, 64)],
                                         rp0[:, osl], start=True, stop=True)
                        nc.tensor.matmul(pp[64:128, :], lhs_p[:, bass.ts(j1, 64)],
                                         rp1[:, osl], start=True, stop=True)
                        pd = psum_pool.tile([128, 512], f32, tag="pd")
                        nc.tensor.matmul(pd[0:64, :], lhs_d[:, bass.ts(j0, 64)],
                                         rd0[:, osl], start=True, stop=True)
                        nc.tensor.matmul(pd[64:128, :], lhs_d[:, bass.ts(j1, 64)],
                                         rd1[:, osl], start=True, stop=True)

                        if _skip == 'noev':
                            if oc == 0:
                                nc.vector.tensor_copy(wo[:, 0:512], pp[:])
                            continue
                        sp = ev_pool.tile([128, 512], bf16, tag="sp")
                        nc.scalar.copy(sp[:], pp[:])
                        sd = ev_pool.tile([128, 512], bf16, tag="sd")
                        nc.scalar.copy(sd[:], pd[:])

                        t1 = ev_pool.tile([128, 512], bf16, tag="t1")
                        nc.vector.tensor_tensor(t1[:], sp[:], gr_p[:, osl], mul)
                        t2 = ev_pool.tile([128, 512], bf16, tag="t2")
                        nc.vector.tensor_tensor(t2[:], sd[:], gr_d[:, osl], mul)
                        t3 = ev_pool.tile([128, 512], f32, tag="t3")
                        if ev_mode == 'subvec':
                            nc.vector.tensor_tensor(t3[:], t1[:], t2[:], sub)
                        else:
                            nc.gpsimd.tensor_tensor(t3[:], t1[:], t2[:], sub)
                        t4 = ev_pool.tile([128, 512], f32, tag="t4")
                        if ev_mode == 'v4gp':
                            nc.gpsimd.tensor_tensor(t4[:], t3[:], wt[:, osl], add)
                        else:
                            nc.vector.tensor_tensor(t4[:], t3[:], wt[:, osl], add)
                        if ev_mode == 'clip_gp':
                            nc.gpsimd.tensor_scalar(wo[:, osl], t4[:], WMAX, 0.0,
                                                    amin, amax)
                        else:
                            nc.vector.tensor_scalar(wo[:, osl], t4[:], WMAX, 0.0,
                                                    amin, amax)
                    (nc.scalar if out_act else nc.sync).dma_start(W_new[s], wo[:])
                emit_traces()

    nc.finalize()
    return nc


def prepare_inputs(Xd, Xpost, Vpost, xbar_pre, u_pot, u_dep, W, A_p, A_d, dmap,
                   fp8_pot=False, fp8_dep=False, blob=False, lhsd_fp8=False,
                   gr_pe=False):
    """Host-side shard + repack.  Returns list of per-core input dicts.

    With fp8_*: the folded dmap*A tensor is stored as float8_e4m3 scaled up
    by FP8_SCALE (A ~ 1e-3 underflows e4m3 subnormals); the matching
    replicated gate tensor is scaled down by 1/FP8_SCALE to compensate
    (exact power-of-two, no precision loss).
    """
    gp = (Xpost * np.maximum(u_pot, 0.0)).astype(F32)        # (B, N)
    gd = np.maximum(u_dep, 0.0).astype(F32)
    FP8 = ml_dtypes.float8_e4m3
    sp = 1.0 / FP8_SCALE if fp8_pot else 1.0
    sd = 1.0 / FP8_SCALE if fp8_dep else 1.0
    if gr_pe:
        grc_p = (gp * sp).astype(BF16)                    # (8, N)
        grc_d = (gd * sd).astype(BF16)
        repm = np.zeros((8, 128), dtype=BF16)
        repm[(np.arange(128) % 64) // 8, np.arange(128)] = 1
        gr_io = dict(GRC_p=grc_p, GRC_d=grc_d, REP=repm)
    else:
        grep_p = np.tile(np.repeat(gp * sp, 8, axis=0), (2, 1)).astype(BF16)
        grep_d = np.tile(np.repeat(gd * sd, 8, axis=0), (2, 1)).astype(BF16)
        gr_io = dict(GR_p=grep_p, GR_d=grep_d)

    if fp8_pot:
        DA_p_full = (dmap * (A_p * FP8_SCALE)[None]).astype(FP8)
    else:
        DA_p_full = (dmap * A_p[None]).astype(BF16)          # (D, N, N)
    if fp8_dep:
        DA_d_full = (dmap * (A_d * FP8_SCALE)[None]).astype(FP8)
    else:
        DA_d_full = (dmap * A_d[None]).astype(BF16)

    xb_flat = xbar_pre.reshape(D * B, N)
    xdec_flat = (np.float32(1.0 - ALPHA_X) * Xd).reshape(D * B, N).astype(F32)
    up_r = u_pot.reshape(128, 128).astype(F32)
    ud_r = u_dep.reshape(128, 128).astype(F32)
    vp1 = (np.float32(1.0 - ALPHA_P) * Vpost).reshape(128, 128).astype(F32)
    vp2 = (np.float32(1.0 - ALPHA_D) * Vpost).reshape(128, 128).astype(F32)

    def pack_lhs(src, dtype):
        # src: (D, B, N) -> per-core (128, 2*NSUP*64) block-diagonal lhsT.
        out = []
        for ci in range(NCORES):
            sl = slice(ci * E, (ci + 1) * E)
            xs = src[:, :, sl].reshape(D, B, 2 * NSUP, 8)    # d, b, j, el
            blk = np.zeros((2 * NSUP, 8, D, B, 8), dtype=F32)  # j, el_k, d, b, el_m
            for el in range(8):
                blk[:, el, :, :, el] = xs[:, :, :, el].transpose(2, 0, 1)
            # k = el*16 + d (el outer), col = j*64 + b*8 + el_m
            lhs = blk.reshape(2 * NSUP, 128, 64).transpose(1, 0, 2).reshape(128, -1)
            out.append(np.ascontiguousarray(lhs).astype(dtype))
        return out

    lhs_p_cores = pack_lhs(xbar_pre, BF16)
    # Xd is binary 0/1 - exact in fp8, halves the lhsT bytes.
    lhs_d_cores = pack_lhs(Xd, FP8 if lhsd_fp8 else BF16)

    in_maps = []
    for ci in range(NCORES):
        sl = slice(ci * E, (ci + 1) * E)

        def pack_da(full):
            # (D, E, N) -> (NSUP, 2, 128, N), k = el*16 + d
            x = full[:, sl].reshape(D, NSUP, 2, 8, N).transpose(1, 2, 3, 0, 4)
            return np.ascontiguousarray(x.reshape(NSUP, 2, 128, N))

        w_c = W[:, sl].reshape(B, NSUP, 2, 8, N).transpose(1, 2, 0, 3, 4)
        w_c = np.ascontiguousarray(w_c.reshape(NSUP, 128, N), dtype=F32)

        if blob:
            pda_p = pack_da(DA_p_full)  # (NSUP, 2, 128, N) fp8
            pda_d = pack_da(DA_d_full)
            bl = np.empty((NSUP, 128, 8 * N), np.uint8)
            bl[:, :, 0 * N:1 * N] = pda_p[:, 0].view(np.uint8)
            bl[:, :, 1 * N:2 * N] = pda_p[:, 1].view(np.uint8)
            bl[:, :, 2 * N:3 * N] = pda_d[:, 0].view(np.uint8)
            bl[:, :, 3 * N:4 * N] = pda_d[:, 1].view(np.uint8)
            bl[:, :, 4 * N:8 * N] = w_c.view(np.uint8)
            io = dict(BLOB=bl)
        else:
            io = dict(DA_p=pack_da(DA_p_full), DA_d=pack_da(DA_d_full), W_in=w_c)

        in_maps.append(dict(
            **io,
            LHS_p=lhs_p_cores[ci], LHS_d=lhs_d_cores[ci],
            **gr_io,
            XB=np.ascontiguousarray(xb_flat[:, sl], dtype=F32),
            XDEC=np.ascontiguousarray(xdec_flat[:, sl]),
            UP=up_r, UD=ud_r, VP1=vp1, VP2=vp2,
        ))
    return in_maps


def assemble_outputs(results, W):
    W_new = np.empty((B, N, N), dtype=F32)
    xbar_new = np.empty((D * B, N), dtype=F32)
    for ci in range(NCORES):
        sl = slice(ci * E, (ci + 1) * E)
        wn = results[ci]["W_new"].reshape(NSUP, 2, B, 8, N).transpose(2, 0, 1, 3, 4)
        W_new[:, sl] = wn.reshape(B, E, N)
        xbar_new[:, sl] = results[ci]["XB_new"]
    u_pot_new = results[0]["UP_new"].reshape(B, N)
    u_dep_new = results[0]["UD_new"].reshape(B, N)
    out = np.asarray(W, dtype=F32)
    return out, W_new, xbar_new.reshape(D, B, N), u_pot_new, u_dep_new


# Production configuration: fp8 folded dmap*A tensors (exact 0/1 dmap,
# power-of-two compensation in the gate tensors), all-DVE elementwise,
# W_new stores issued from the ACT HWDGE ring.
KERNEL_CFG = dict(fp8_pot=True, fp8_dep=True, ev_mode="wide", out_act=True,
                  lhsd_fp8=True, gr_pe=True)

_NC_CACHE: dict = {}


def _get_nc():
    if "nc" not in _NC_CACHE:
        _NC_CACHE["nc"] = build_kernel(**KERNEL_CFG)
    return _NC_CACHE["nc"]


def kernel(Xd, Xpost, Vpost, xbar_pre, u_pot, u_dep, W, A_p, A_d, dmap):
    args = [np.asarray(a, dtype=F32) for a in
            (Xd, Xpost, Vpost, xbar_pre, u_pot, u_dep, W, A_p, A_d, dmap)]
    in_maps = prepare_inputs(*args, fp8_pot=KERNEL_CFG["fp8_pot"],
                             fp8_dep=KERNEL_CFG["fp8_dep"],
                             lhsd_fp8=KERNEL_CFG["lhsd_fp8"],
                             gr_pe=KERNEL_CFG["gr_pe"])
    nc = _get_nc()
    # The axon-tunneled device occasionally reports a transient
    # NRT_EXEC_UNIT_UNRECOVERABLE that clears after the remote worker
    # restarts; retry a couple of times before giving up.
    last_exc = None
    for attempt in range(3):
        try:
            res = run_bass_kernel_spmd(nc, in_maps, core_ids=list(range(NCORES)))
            return assemble_outputs(res.results, args[6])
        except Exception as exc:  # noqa: BLE001
            last_exc = exc
            if attempt < 2:
                import time
                time.sleep(45)
    raise last_exc


# revision 48
# speedup vs baseline: 1.0086x; 1.0040x over previous
"""Trainium2 Bass kernel for the Clopath plasticity rule (nn_Clopath).

Math (reference):
    dW_pot[b,e,o] = sum_d xbar[d,b,e] * dmap[d,e,o] * A_p[e,o] * gp[b,o]
    dW_dep[b,e,o] = sum_d Xd[d,b,e]   * dmap[d,e,o] * A_d[e,o] * gd[b,o]
        gp = Xpost * relu(u_pot),  gd = relu(u_dep)
    W_new = clip(W + dW_pot - dW_dep, 0, 2)
    out = W (pre-update);  plus three exponential trace updates.

Strategy (8 NeuronCores, shard pre-synaptic dim e; DMA-bound at ~161 us/core
per the instruction cost model, vs a ~149 us pure-bytes floor):
  * Host folds A into dmap:  DA_p = dmap*A_p, DA_d = dmap*A_d, stored as
    float8_e4m3 scaled by 256 (A ~ 1e-3 underflows e4m3; the 1/256
    compensation goes into the bf16 gate tensors - exact power of two).
    dmap is 0/1 so fp8 quantizes only A (~6% worst-case on a term that is
    ~1e-4 of W; measured absmax error 1.2e-4 relative on W_new).
  * The d-contraction runs on TensorE as block-diagonal packed matmuls:
    contraction K = 8 e's x 16 d = 128, out M = 64 = (b x e_local), N = 512
    o-chunk.  lhsT carries xbar (bf16) / Xd (fp8, exact 0/1) values on the
    block diagonal (built on host); rhs is a (128, 2048) repack of DA rows.
    Two matmuls (two e-halves) fill a (128, 512) PSUM tile at partition
    offsets 0/64.
  * Gating by gp/gd uses partition-replicated (128, N) bf16 tiles; ScalarE
    evacuates PSUM to bf16, VectorE does the gate multiplies (bf16 2x
    mode), pot-dep subtract, fp32 W add, and a fused min/max clip; W_new
    stores issue from the ACT HWDGE ring to overlap with SP-ring loads.
  * Trace updates run on-device as single fused scalar_tensor_tensor ops.
  * All elementwise/PE work hides under the DMA stream (W fp32 in+out
    33.6 MB/core + DA fp8 16.8 MB/core dominate).
"""

import numpy as np
import ml_dtypes

import concourse.bass as bass
import concourse.bacc as bacc
import concourse.mybir as mybir
from concourse.tile import TileContext
from concourse.bass_utils import run_bass_kernel_spmd

BF16 = ml_dtypes.bfloat16
F32 = np.float32

D, B, N = 16, 8, 2048
NCORES = 8
E = N // NCORES          # 256 pre-synaptic neurons per core
NSUP = E // 16           # 16 supers (16 e's each) per core
ALPHA_X, ALPHA_P, ALPHA_D = 0.95, 0.9, 0.8
WMAX = 2.0
OC = N // 512            # 4 o-chunks of 512


FP8_SCALE = 256.0


def build_kernel(repeat: int = 1, fp8_pot: bool = False, fp8_dep: bool = False,
                 rhs_bufs: int = 2, ev_bufs: int = 3, blob: bool = False,
                 ev_mode: str = 'default', dma_split: bool = False,
                 out_act: bool = False, w_act: bool = False,
                 psum_bufs: int = 3, lhsd_fp8: bool = False,
                 rd_act: bool = False, rd_gp: bool = False,
                 w_gp: bool = False, _tail_split: int = 16,
                 gr_pe: bool = False, _skip: str = '') -> bass.Bass:
    if blob:
        assert fp8_pot and fp8_dep, "blob layout assumes fp8 DA tensors" 
    # Bacc (not plain Bass): its finalize() runs move_matmul_waits_to_
    # ldweights + generate_event_semaphores, which split multi-sem waits to
    # satisfy the 1-wait-per-instruction TRN2 codegen constraint.
    nc = bacc.Bacc()
    dt = mybir.dt
    f32, bf16 = dt.float32, dt.bfloat16
    dt_p = dt.float8e4 if fp8_pot else bf16
    dt_d = dt.float8e4 if fp8_dep else bf16

    # Per-core inputs (host pre-packed into DMA-friendly layouts).
    if blob:
        # One byte-interleaved tensor per super: per partition row =
        # [DA_p h0 | DA_p h1 | DA_d h0 | DA_d h1 (fp8, N bytes each) | W (f32, 4N bytes)]
        BLOB = nc.dram_tensor("BLOB", [NSUP, 128, 8 * N], dt.uint8,
                              kind="ExternalInput")
    else:
        DA_p = nc.dram_tensor("DA_p", [NSUP, 2, 128, N], dt_p, kind="ExternalInput")
        DA_d = nc.dram_tensor("DA_d", [NSUP, 2, 128, N], dt_d, kind="ExternalInput")
        W_in = nc.dram_tensor("W_in", [NSUP, 128, N], f32, kind="ExternalInput")
    LHS_p = nc.dram_tensor("LHS_p", [128, 2 * NSUP * 64], bf16, kind="ExternalInput")
    dt_ld = dt.float8e4 if lhsd_fp8 else bf16
    LHS_d = nc.dram_tensor("LHS_d", [128, 2 * NSUP * 64], dt_ld, kind="ExternalInput")
    if gr_pe:
        GRC_p = nc.dram_tensor("GRC_p", [8, N], bf16, kind="ExternalInput")
        GRC_d = nc.dram_tensor("GRC_d", [8, N], bf16, kind="ExternalInput")
        REP = nc.dram_tensor("REP", [8, 128], bf16, kind="ExternalInput")
    else:
        GR_p = nc.dram_tensor("GR_p", [128, N], bf16, kind="ExternalInput")
        GR_d = nc.dram_tensor("GR_d", [128, N], bf16, kind="ExternalInput")
    XB = nc.dram_tensor("XB", [128, E], f32, kind="ExternalInput")      # xbar slice
    XDEC = nc.dram_tensor("XDEC", [128, E], f32, kind="ExternalInput")  # (1-ax)*Xd
    # Per-core o-slice of the u/V traces, packed column-wise on partitions
    # 0:8 (engine APs must start at partition 0/32/64/96):
    # cols = [u_pot | (1-ap)*V | u_dep | (1-ad)*V], each E wide.
    UV = nc.dram_tensor("UV", [8, 4 * E], f32, kind="ExternalInput")

    W_new = nc.dram_tensor("W_new", [NSUP, 128, N], f32, kind="ExternalOutput")
    XB_new = nc.dram_tensor("XB_new", [128, E], f32, kind="ExternalOutput")
    UVN = nc.dram_tensor("UVN", [8, 2 * E], f32, kind="ExternalOutput")

    mul = mybir.AluOpType.mult
    add = mybir.AluOpType.add
    sub = mybir.AluOpType.subtract
    amin = mybir.AluOpType.min
    amax = mybir.AluOpType.max

    with TileContext(nc) as tc:
        with (
            tc.tile_pool(name="const", bufs=1) as cpool,
            tc.tile_pool(name="rhs", bufs=rhs_bufs) as rhs_pool,
            tc.tile_pool(name="wio", bufs=2) as w_pool,
            tc.tile_pool(name="ev", bufs=ev_bufs) as ev_pool,
            # wide mode uses (128, 1024) = 2-bank PSUM tiles; 2 tags x 2 bufs
            # x 2 banks = all 8 banks.
            tc.tile_pool(name="psum",
                         bufs=(2 if ev_mode in ('wide', 'wide2') else psum_bufs),
                         space="PSUM") as psum_pool,
        ):
            # One-time constant loads.  In 'wide2' mode the slices needed by
            # the first super load first as separate small tiles so the first
            # compute chain starts after ~200 KB instead of ~2.2 MB.
            fine = ev_mode == 'wide2'
            if fine:
                lhs_p0 = cpool.tile([128, 128], bf16, tag="lhs_p0")
                nc.scalar.dma_start(lhs_p0[:], LHS_p[:, 0:128])
                lhs_d0 = cpool.tile([128, 128], dt_ld, tag="lhs_d0")
                nc.scalar.dma_start(lhs_d0[:], LHS_d[:, 0:128])
                gr_p0 = cpool.tile([128, 1024], bf16, tag="gr_p0")
                nc.scalar.dma_start(gr_p0[:], GR_p[:, 0:1024])
                gr_d0 = cpool.tile([128, 1024], bf16, tag="gr_d0")
                nc.scalar.dma_start(gr_d0[:], GR_d[:, 0:1024])
            head = False  # head reorder measured neutral; stream-paced
            h_tiles = {}
            if head:
                # Interleave super-0's loads with the constants in dependency
                # order, so the first matmul->evac->gate chain starts after
                # ~1 MB instead of ~2.3 MB of DMA.
                lhs_p = cpool.tile([128, 2 * NSUP * 64], bf16, tag="lhs_p")
                nc.sync.dma_start(lhs_p[:], LHS_p[:])
                h_tiles['rp0'] = rhs_pool.tile([128, N], dt_p, tag="rp0", name="h_rp0")
                nc.sync.dma_start(h_tiles['rp0'][:], DA_p[0, 0])
                h_tiles['rp1'] = rhs_pool.tile([128, N], dt_p, tag="rp1", name="h_rp1")
                nc.sync.dma_start(h_tiles['rp1'][:], DA_p[0, 1])
                gr_p = cpool.tile([128, N], bf16, tag="gr_p")
                nc.sync.dma_start(gr_p[:], GR_p[:])
                lhs_d = cpool.tile([128, 2 * NSUP * 64], dt_ld, tag="lhs_d")
                nc.sync.dma_start(lhs_d[:], LHS_d[:])
                h_tiles['rd0'] = rhs_pool.tile([128, N], dt_d, tag="rd0", name="h_rd0")
                nc.sync.dma_start(h_tiles['rd0'][:], DA_d[0, 0])
                h_tiles['rd1'] = rhs_pool.tile([128, N], dt_d, tag="rd1", name="h_rd1")
                nc.sync.dma_start(h_tiles['rd1'][:], DA_d[0, 1])
                gr_d = cpool.tile([128, N], bf16, tag="gr_d")
                nc.sync.dma_start(gr_d[:], GR_d[:])
                h_tiles['wt'] = w_pool.tile([128, N], f32, tag="wt", name="h_wt")
                nc.sync.dma_start(h_tiles['wt'][:], W_in[0])
            else:
                lhs_p = cpool.tile([128, 2 * NSUP * 64], bf16, tag="lhs_p")
                nc.sync.dma_start(lhs_p[:], LHS_p[:])
                lhs_d = cpool.tile([128, 2 * NSUP * 64], dt_ld, tag="lhs_d")
                nc.sync.dma_start(lhs_d[:], LHS_d[:])
                gr_p = cpool.tile([128, N], bf16, tag="gr_p")
                gr_d = cpool.tile([128, N], bf16, tag="gr_d")
                if gr_pe:
                    # Build the partition-replicated gate tensors on-chip:
                    # PE multiplies the compact (8, N) rows by a 0/1 selector
                    # (exact, x1.0 accumulate) - saves ~1 MB of HBM traffic.
                    grc_p = cpool.tile([8, N], bf16, tag="grc_p")
                    nc.scalar.dma_start(grc_p[:], GRC_p[:])
                    grc_d = cpool.tile([8, N], bf16, tag="grc_d")
                    nc.scalar.dma_start(grc_d[:], GRC_d[:])
                    rept = cpool.tile([8, 128], bf16, tag="rept")
                    nc.scalar.dma_start(rept[:], REP[:])
                    for gsrc, gdst, ptag in ((grc_p, gr_p, "pp"), (grc_d, gr_d, "pd")):
                        for h in range(N // 1024):
                            rt = psum_pool.tile([128, 1024], f32, tag=ptag)
                            for q in range(2):
                                c0 = h * 1024 + q * 512
                                nc.tensor.matmul(rt[:, q * 512:(q + 1) * 512],
                                                 rept[:], gsrc[:, c0:c0 + 512],
                                                 start=True, stop=True)
                            nc.scalar.copy(gdst[:, h * 1024:(h + 1) * 1024], rt[:])
                else:
                    # gate tensors via the ACT ring: it is store-only otherwise,
                    # so its sequencer is idle at t=0 and the transfers overlap
                    # the SP ring's constant loads.
                    nc.scalar.dma_start(gr_p[:], GR_p[:])
                    nc.scalar.dma_start(gr_d[:], GR_d[:])

            def emit_traces():
                # Trace updates (tiny).
                xb = ev_pool.tile([128, E], f32, tag="xb")
                nc.sync.dma_start(xb[:], XB[:])
                xd = ev_pool.tile([128, E], f32, tag="xd")
                nc.sync.dma_start(xd[:], XDEC[:])
                xn = ev_pool.tile([128, E], f32, tag="xn")
                nc.vector.scalar_tensor_tensor(xn[:], xb[:], ALPHA_X, xd[:], mul, add)
                nc.sync.dma_start(XB_new[:], xn[:])

                uv = ev_pool.tile([8, 4 * E], f32, tag="uv")
                nc.sync.dma_start(uv[:], UV[:])
                uvn = ev_pool.tile([8, 2 * E], f32, tag="uvn")
                nc.vector.scalar_tensor_tensor(uvn[:, 0:E], uv[:, 0:E],
                                               ALPHA_P, uv[:, E:2 * E], mul, add)
                nc.vector.scalar_tensor_tensor(uvn[:, E:2 * E], uv[:, 2 * E:3 * E],
                                               ALPHA_D, uv[:, 3 * E:4 * E], mul, add)
                nc.sync.dma_start(UVN[:], uvn[:])

            for r in range(repeat):
                for s in range(NSUP):
                    if head and r == 0 and s == 0:
                        rp0, rp1 = h_tiles['rp0'], h_tiles['rp1']
                        rd0, rd1 = h_tiles['rd0'], h_tiles['rd1']
                        wt = h_tiles['wt']
                    elif ev_mode == 'wide2' and s == 0:
                        rp0 = rp1 = rd0 = rd1 = wt = None  # fine-grained below
                    elif blob:
                        bt = rhs_pool.tile([128, 8 * N], dt.uint8, tag="bt")
                        nc.sync.dma_start(bt[:], BLOB[s])
                        rp0 = bt[:, 0 * N:1 * N].bitcast(dt_p)
                        rp1 = bt[:, 1 * N:2 * N].bitcast(dt_p)
                        rd0 = bt[:, 2 * N:3 * N].bitcast(dt_d)
                        rd1 = bt[:, 3 * N:4 * N].bitcast(dt_d)
                        wt = bt[:, 4 * N:8 * N].bitcast(f32)
                    else:
                        rp0 = rhs_pool.tile([128, N], dt_p, tag="rp0")
                        rp1 = rhs_pool.tile([128, N], dt_p, tag="rp1")
                        rd0 = rhs_pool.tile([128, N], dt_d, tag="rd0")
                        rd1 = rhs_pool.tile([128, N], dt_d, tag="rd1")
                        wt = w_pool.tile([128, N], f32, tag="wt")
                        if dma_split:
                            h = N // 2
                            for t_, src in ((rp0, DA_p[s, 0]), (rp1, DA_p[s, 1]),
                                            (rd0, DA_d[s, 0]), (rd1, DA_d[s, 1]),
                                            (wt, W_in[s])):
                                nc.sync.dma_start(t_[:, 0:h], src[:, 0:h])
                                nc.sync.dma_start(t_[:, h:N], src[:, h:N])
                        else:
                            nc.sync.dma_start(rp0[:], DA_p[s, 0])
                            nc.sync.dma_start(rp1[:], DA_p[s, 1])
                            rd_eng = nc.gpsimd if rd_gp else (nc.scalar if rd_act else nc.sync)
                            rd_eng.dma_start(rd0[:], DA_d[s, 0])
                            rd_eng.dma_start(rd1[:], DA_d[s, 1])
                            (nc.gpsimd if w_gp else (nc.scalar if w_act else nc.sync)).dma_start(wt[:], W_in[s])
                    wo = w_pool.tile([128, N], f32, tag="wo")

                    j0, j1 = 2 * s, 2 * s + 1
                    if _skip == 'dmaonly':
                        nc.vector.tensor_copy(wo[:, 0:512], wt[:, 0:512])
                        nc.sync.dma_start(W_new[s], wo[:])
                        continue
                    if ev_mode == 'wide2' and s == 0:
                        # First super at oc granularity with per-chunk DMAs
                        # into separate tiles: the first matmul/DVE chain
                        # starts as soon as its ~200 KB lands.
                        for oc in range(OC):
                            osl = bass.ts(oc, 512)
                            fr0 = rhs_pool.tile([128, 512], dt_p, tag="f0")
                            fr1 = rhs_pool.tile([128, 512], dt_p, tag="f1")
                            fr2 = rhs_pool.tile([128, 512], dt_d, tag="f2")
                            fr3 = rhs_pool.tile([128, 512], dt_d, tag="f3")
                            nc.scalar.dma_start(fr0[:], DA_p[0, 0][:, osl])
                            nc.scalar.dma_start(fr1[:], DA_p[0, 1][:, osl])
                            nc.scalar.dma_start(fr2[:], DA_d[0, 0][:, osl])
                            nc.scalar.dma_start(fr3[:], DA_d[0, 1][:, osl])
                            fw = rhs_pool.tile([128, 512], f32, tag="fw")
                            nc.scalar.dma_start(fw[:], W_in[0][:, osl])
                            pp = psum_pool.tile([128, 512], f32, tag="pp")
                            nc.tensor.matmul(pp[0:64, :], lhs_p0[:, 0:64],
                                             fr0[:], start=True, stop=True)
                            nc.tensor.matmul(pp[64:128, :], lhs_p0[:, 64:128],
                                             fr1[:], start=True, stop=True)
                            pd = psum_pool.tile([128, 512], f32, tag="pd")
                            nc.tensor.matmul(pd[0:64, :], lhs_d0[:, 0:64],
                                             fr2[:], start=True, stop=True)
                            nc.tensor.matmul(pd[64:128, :], lhs_d0[:, 64:128],
                                             fr3[:], start=True, stop=True)
                            sp = ev_pool.tile([128, 512], bf16, tag="fsp")
                            nc.scalar.copy(sp[:], pp[:])
                            sd = ev_pool.tile([128, 512], bf16, tag="fsd")
                            nc.scalar.copy(sd[:], pd[:])
                            gp_src = gr_p0 if oc < 2 else gr_p
                            gd_src = gr_d0 if oc < 2 else gr_d
                            t1 = ev_pool.tile([128, 512], bf16, tag="ft1")
                            nc.vector.tensor_tensor(t1[:], sp[:], gp_src[:, osl], mul)
                            t2 = ev_pool.tile([128, 512], bf16, tag="ft2")
                            nc.vector.tensor_tensor(t2[:], sd[:], gd_src[:, osl], mul)
                            t3 = ev_pool.tile([128, 512], f32, tag="ft3")
                            nc.vector.tensor_tensor(t3[:], t1[:], t2[:], sub)
                            t4 = ev_pool.tile([128, 512], f32, tag="ft4")
                            nc.vector.tensor_tensor(t4[:], t3[:], fw[:], add)
                            nc.vector.tensor_scalar(wo[:, osl], t4[:], WMAX, 0.0,
                                                    amin, amax)
                        (nc.scalar if out_act else nc.sync).dma_start(W_new[0], wo[:])
                        continue
                    narrow_tail = False  # measured worse: narrow DVE ops cost more than the shorter tail chain saves
                    if ev_mode in ('wide', 'wide2'):
                        # oc-pair granularity: (128, 1024) PSUM tiles (2 banks),
                        # one ScalarE evacuation + 1024-wide DVE ops per pair -
                        # amortizes the per-op fixed cost with identical math.
                        last = (ev_mode == 'wide2' and s == NSUP - 1) or \
                               (ev_mode == 'wide' and s >= NSUP - _tail_split)
                        # split@1024 stores release DMA work at finer grain;
                        # best in sweep: split every super
                        for ocp in range(OC // 2):
                            pp = psum_pool.tile([128, 1024], f32, tag="pp")
                            pd = psum_pool.tile([128, 1024], f32, tag="pd")
                            for w in range(2):
                                oc = 2 * ocp + w
                                osl = bass.ts(oc, 512)
                                wsl = slice(w * 512, (w + 1) * 512)
                                nc.tensor.matmul(pp[0:64, wsl], lhs_p[:, bass.ts(j0, 64)],
                                                 rp0[:, osl], start=True, stop=True)
                                nc.tensor.matmul(pp[64:128, wsl], lhs_p[:, bass.ts(j1, 64)],
                                                 rp1[:, osl], start=True, stop=True)
                                nc.tensor.matmul(pd[0:64, wsl], lhs_d[:, bass.ts(j0, 64)],
                                                 rd0[:, osl], start=True, stop=True)
                                nc.tensor.matmul(pd[64:128, wsl], lhs_d[:, bass.ts(j1, 64)],
                                                 rd1[:, osl], start=True, stop=True)
                            psl = bass.ts(ocp, 1024)
                            sp = ev_pool.tile([128, 1024], bf16, tag="sp")
                            nc.scalar.copy(sp[:], pp[:])
                            sd = ev_pool.tile([128, 1024], bf16, tag="sd")
                            nc.scalar.copy(sd[:], pd[:])
                            t1 = ev_pool.tile([128, 1024], bf16, tag="t1")
                            nc.vector.tensor_tensor(t1[:], sp[:], gr_p[:, psl], mul)
                            t2 = ev_pool.tile([128, 1024], bf16, tag="t2")
                            nc.vector.tensor_tensor(t2[:], sd[:], gr_d[:, psl], mul)
                            t3 = ev_pool.tile([128, 1024], f32, tag="t3")
                            nc.vector.tensor_tensor(t3[:], t1[:], t2[:], sub)
                            t4 = ev_pool.tile([128, 1024], f32, tag="t4")
                            nc.vector.tensor_tensor(t4[:], t3[:], wt[:, psl], add)
                            nc.vector.tensor_scalar(wo[:, psl], t4[:], WMAX, 0.0,
                                                    amin, amax)
                            if last:
                                # split stores so the final chunk's store is
                                # the only thing on the kernel tail
                                (nc.scalar if out_act else nc.sync).dma_start(
                                    W_new[s][:, psl], wo[:, psl])
                        if not last:
                            (nc.scalar if out_act else nc.sync).dma_start(
                                W_new[s], wo[:])
                        continue
                    for oc in range(OC):
                        osl = bass.ts(oc, 512)
                        pp = psum_pool.tile([128, 512], f32, tag="pp")
                        nc.tensor.matmul(pp[0:64, :], lhs_p[:, bass.ts(j0, 64)],
                                         rp0[:, osl], start=True, stop=True)
                        nc.tensor.matmul(pp[64:128, :], lhs_p[:, bass.ts(j1, 64)],
                                         rp1[:, osl], start=True, stop=True)
                        pd = psum_pool.tile([128, 512], f32, tag="pd")
                        nc.tensor.matmul(pd[0:64, :], lhs_d[:, bass.ts(j0, 64)],
                                         rd0[:, osl], start=True, stop=True)
                        nc.tensor.matmul(pd[64:128, :], lhs_d[:, bass.ts(j1, 64)],
                                         rd1[:, osl], start=True, stop=True)

                        if _skip == 'noev':
                            if oc == 0:
                                nc.vector.tensor_copy(wo[:, 0:512], pp[:])
                            continue
                        sp = ev_pool.tile([128, 512], bf16, tag="sp")
                        nc.scalar.copy(sp[:], pp[:])
                        sd = ev_pool.tile([128, 512], bf16, tag="sd")
                        nc.scalar.copy(sd[:], pd[:])

                        t1 = ev_pool.tile([128, 512], bf16, tag="t1")
                        nc.vector.tensor_tensor(t1[:], sp[:], gr_p[:, osl], mul)
                        t2 = ev_pool.tile([128, 512], bf16, tag="t2")
                        nc.vector.tensor_tensor(t2[:], sd[:], gr_d[:, osl], mul)
                        t3 = ev_pool.tile([128, 512], f32, tag="t3")
                        if ev_mode == 'subvec':
                            nc.vector.tensor_tensor(t3[:], t1[:], t2[:], sub)
                        else:
                            nc.gpsimd.tensor_tensor(t3[:], t1[:], t2[:], sub)
                        t4 = ev_pool.tile([128, 512], f32, tag="t4")
                        if ev_mode == 'v4gp':
                            nc.gpsimd.tensor_tensor(t4[:], t3[:], wt[:, osl], add)
                        else:
                            nc.vector.tensor_tensor(t4[:], t3[:], wt[:, osl], add)
                        if ev_mode == 'clip_gp':
                            nc.gpsimd.tensor_scalar(wo[:, osl], t4[:], WMAX, 0.0,
                                                    amin, amax)
                        else:
                            nc.vector.tensor_scalar(wo[:, osl], t4[:], WMAX, 0.0,
                                                    amin, amax)
                    (nc.scalar if out_act else nc.sync).dma_start(W_new[s], wo[:])
                emit_traces()

    nc.finalize()
    return nc


def prepare_inputs(Xd, Xpost, Vpost, xbar_pre, u_pot, u_dep, W, A_p, A_d, dmap,
                   fp8_pot=False, fp8_dep=False, blob=False, lhsd_fp8=False,
                   gr_pe=False):
    """Host-side shard + repack.  Returns list of per-core input dicts.

    With fp8_*: the folded dmap*A tensor is stored as float8_e4m3 scaled up
    by FP8_SCALE (A ~ 1e-3 underflows e4m3 subnormals); the matching
    replicated gate tensor is scaled down by 1/FP8_SCALE to compensate
    (exact power-of-two, no precision loss).
    """
    gp = (Xpost * np.maximum(u_pot, 0.0)).astype(F32)        # (B, N)
    gd = np.maximum(u_dep, 0.0).astype(F32)
    FP8 = ml_dtypes.float8_e4m3
    sp = 1.0 / FP8_SCALE if fp8_pot else 1.0
    sd = 1.0 / FP8_SCALE if fp8_dep else 1.0
    if gr_pe:
        grc_p = (gp * sp).astype(BF16)                    # (8, N)
        grc_d = (gd * sd).astype(BF16)
        repm = np.zeros((8, 128), dtype=BF16)
        repm[(np.arange(128) % 64) // 8, np.arange(128)] = 1
        gr_io = dict(GRC_p=grc_p, GRC_d=grc_d, REP=repm)
    else:
        grep_p = np.tile(np.repeat(gp * sp, 8, axis=0), (2, 1)).astype(BF16)
        grep_d = np.tile(np.repeat(gd * sd, 8, axis=0), (2, 1)).astype(BF16)
        gr_io = dict(GR_p=grep_p, GR_d=grep_d)

    if fp8_pot:
        DA_p_full = (dmap * (A_p * FP8_SCALE)[None]).astype(FP8)
    else:
        DA_p_full = (dmap * A_p[None]).astype(BF16)          # (D, N, N)
    if fp8_dep:
        DA_d_full = (dmap * (A_d * FP8_SCALE)[None]).astype(FP8)
    else:
        DA_d_full = (dmap * A_d[None]).astype(BF16)

    xb_flat = xbar_pre.reshape(D * B, N)
    xdec_flat = (np.float32(1.0 - ALPHA_X) * Xd).reshape(D * B, N).astype(F32)
    vp1 = (np.float32(1.0 - ALPHA_P) * Vpost).astype(F32)
    vp2 = (np.float32(1.0 - ALPHA_D) * Vpost).astype(F32)

    def pack_lhs(src, dtype):
        # src: (D, B, N) -> per-core (128, 2*NSUP*64) block-diagonal lhsT.
        out = []
        for ci in range(NCORES):
            sl = slice(ci * E, (ci + 1) * E)
            xs = src[:, :, sl].reshape(D, B, 2 * NSUP, 8)    # d, b, j, el
            blk = np.zeros((2 * NSUP, 8, D, B, 8), dtype=F32)  # j, el_k, d, b, el_m
            for el in range(8):
                blk[:, el, :, :, el] = xs[:, :, :, el].transpose(2, 0, 1)
            # k = el*16 + d (el outer), col = j*64 + b*8 + el_m
            lhs = blk.reshape(2 * NSUP, 128, 64).transpose(1, 0, 2).reshape(128, -1)
            out.append(np.ascontiguousarray(lhs).astype(dtype))
        return out

    lhs_p_cores = pack_lhs(xbar_pre, BF16)
    # Xd is binary 0/1 - exact in fp8, halves the lhsT bytes.
    lhs_d_cores = pack_lhs(Xd, FP8 if lhsd_fp8 else BF16)

    in_maps = []
    for ci in range(NCORES):
        sl = slice(ci * E, (ci + 1) * E)

        def pack_da(full):
            # (D, E, N) -> (NSUP, 2, 128, N), k = el*16 + d
            x = full[:, sl].reshape(D, NSUP, 2, 8, N).transpose(1, 2, 3, 0, 4)
            return np.ascontiguousarray(x.reshape(NSUP, 2, 128, N))

        w_c = W[:, sl].reshape(B, NSUP, 2, 8, N).transpose(1, 2, 0, 3, 4)
        w_c = np.ascontiguousarray(w_c.reshape(NSUP, 128, N), dtype=F32)

        if blob:
            pda_p = pack_da(DA_p_full)  # (NSUP, 2, 128, N) fp8
            pda_d = pack_da(DA_d_full)
            bl = np.empty((NSUP, 128, 8 * N), np.uint8)
            bl[:, :, 0 * N:1 * N] = pda_p[:, 0].view(np.uint8)
            bl[:, :, 1 * N:2 * N] = pda_p[:, 1].view(np.uint8)
            bl[:, :, 2 * N:3 * N] = pda_d[:, 0].view(np.uint8)
            bl[:, :, 3 * N:4 * N] = pda_d[:, 1].view(np.uint8)
            bl[:, :, 4 * N:8 * N] = w_c.view(np.uint8)
            io = dict(BLOB=bl)
        else:
            io = dict(DA_p=pack_da(DA_p_full), DA_d=pack_da(DA_d_full), W_in=w_c)

        in_maps.append(dict(
            **io,
            LHS_p=lhs_p_cores[ci], LHS_d=lhs_d_cores[ci],
            **gr_io,
            XB=np.ascontiguousarray(xb_flat[:, sl], dtype=F32),
            XDEC=np.ascontiguousarray(xdec_flat[:, sl]),
            UV=np.concatenate([u_pot[:, sl], vp1[:, sl],
                               u_dep[:, sl], vp2[:, sl]], axis=1).astype(F32),
        ))
    return in_maps


def assemble_outputs(results, W):
    W_new = np.empty((B, N, N), dtype=F32)
    xbar_new = np.empty((D * B, N), dtype=F32)
    for ci in range(NCORES):
        sl = slice(ci * E, (ci + 1) * E)
        wn = results[ci]["W_new"].reshape(NSUP, 2, B, 8, N).transpose(2, 0, 1, 3, 4)
        W_new[:, sl] = wn.reshape(B, E, N)
        xbar_new[:, sl] = results[ci]["XB_new"]
    u_pot_new = np.empty((B, N), dtype=F32)
    u_dep_new = np.empty((B, N), dtype=F32)
    for ci in range(NCORES):
        sl = slice(ci * E, (ci + 1) * E)
        u_pot_new[:, sl] = results[ci]["UVN"][:, 0:E]
        u_dep_new[:, sl] = results[ci]["UVN"][:, E:2 * E]
    out = np.asarray(W, dtype=F32)
    return out, W_new, xbar_new.reshape(D, B, N), u_pot_new, u_dep_new


# Production configuration: fp8 folded dmap*A tensors (exact 0/1 dmap,
# power-of-two compensation in the gate tensors), all-DVE elementwise,
# W_new stores issued from the ACT HWDGE ring.
KERNEL_CFG = dict(fp8_pot=True, fp8_dep=True, ev_mode="wide", out_act=True,
                  lhsd_fp8=True, gr_pe=True)

_NC_CACHE: dict = {}


def _get_nc():
    if "nc" not in _NC_CACHE:
        _NC_CACHE["nc"] = build_kernel(**KERNEL_CFG)
    return _NC_CACHE["nc"]


def kernel(Xd, Xpost, Vpost, xbar_pre, u_pot, u_dep, W, A_p, A_d, dmap):
    args = [np.asarray(a, dtype=F32) for a in
            (Xd, Xpost, Vpost, xbar_pre, u_pot, u_dep, W, A_p, A_d, dmap)]
    in_maps = prepare_inputs(*args, fp8_pot=KERNEL_CFG["fp8_pot"],
                             fp8_dep=KERNEL_CFG["fp8_dep"],
                             lhsd_fp8=KERNEL_CFG["lhsd_fp8"],
                             gr_pe=KERNEL_CFG["gr_pe"])
    nc = _get_nc()
    # The axon-tunneled device occasionally reports a transient
    # NRT_EXEC_UNIT_UNRECOVERABLE that clears after the remote worker
    # restarts; retry a couple of times before giving up.
    last_exc = None
    for attempt in range(3):
        try:
            res = run_bass_kernel_spmd(nc, in_maps, core_ids=list(range(NCORES)))
            return assemble_outputs(res.results, args[6])
        except Exception as exc:  # noqa: BLE001
            last_exc = exc
            if attempt < 2:
                import time
                time.sleep(45)
    raise last_exc


# revision 50
# speedup vs baseline: 1.0394x; 1.0306x over previous
"""Trainium2 Bass kernel for the Clopath plasticity rule (nn_Clopath).

Math (reference):
    dW_pot[b,e,o] = sum_d xbar[d,b,e] * dmap[d,e,o] * A_p[e,o] * gp[b,o]
    dW_dep[b,e,o] = sum_d Xd[d,b,e]   * dmap[d,e,o] * A_d[e,o] * gd[b,o]
        gp = Xpost * relu(u_pot),  gd = relu(u_dep)
    W_new = clip(W + dW_pot - dW_dep, 0, 2)
    out = W (pre-update);  plus three exponential trace updates.

Strategy (8 NeuronCores, shard pre-synaptic dim e; DMA-bound at ~161 us/core
per the instruction cost model, vs a ~149 us pure-bytes floor):
  * Host folds A into dmap:  DA_p = dmap*A_p, DA_d = dmap*A_d, stored as
    float8_e4m3 scaled by 256 (A ~ 1e-3 underflows e4m3; the 1/256
    compensation goes into the bf16 gate tensors - exact power of two).
    dmap is 0/1 so fp8 quantizes only A (~6% worst-case on a term that is
    ~1e-4 of W; measured absmax error 1.2e-4 relative on W_new).
  * The d-contraction runs on TensorE as block-diagonal packed matmuls:
    contraction K = 8 e's x 16 d = 128, out M = 64 = (b x e_local), N = 512
    o-chunk.  lhsT carries xbar (bf16) / Xd (fp8, exact 0/1) values on the
    block diagonal (built on host); rhs is a (128, 2048) repack of DA rows.
    Two matmuls (two e-halves) fill a (128, 512) PSUM tile at partition
    offsets 0/64.
  * Gating by gp/gd uses partition-replicated (128, N) bf16 tiles; ScalarE
    evacuates PSUM to bf16, VectorE does the gate multiplies (bf16 2x
    mode), pot-dep subtract, fp32 W add, and a fused min/max clip; W_new
    stores issue from the ACT HWDGE ring to overlap with SP-ring loads.
  * Trace updates run on-device as single fused scalar_tensor_tensor ops.
  * All elementwise/PE work hides under the DMA stream (W fp32 in+out
    33.6 MB/core + DA fp8 16.8 MB/core dominate).
"""

import numpy as np
import ml_dtypes

import concourse.bass as bass
import concourse.bacc as bacc
import concourse.mybir as mybir
from concourse.tile import TileContext
from concourse.bass_utils import run_bass_kernel_spmd

BF16 = ml_dtypes.bfloat16
F32 = np.float32

D, B, N = 16, 8, 2048
NCORES = 8
E = N // NCORES          # 256 pre-synaptic neurons per core
NSUP = E // 16           # 16 supers (16 e's each) per core
ALPHA_X, ALPHA_P, ALPHA_D = 0.95, 0.9, 0.8
WMAX = 2.0
OC = N // 512            # 4 o-chunks of 512


FP8_SCALE = 256.0


def build_kernel(repeat: int = 1, fp8_pot: bool = False, fp8_dep: bool = False,
                 rhs_bufs: int = 2, ev_bufs: int = 3, blob: bool = False,
                 ev_mode: str = 'default', dma_split: bool = False,
                 out_act: bool = False, w_act: bool = False,
                 psum_bufs: int = 3, lhsd_fp8: bool = False,
                 rd_act: bool = False, rd_gp: bool = False,
                 w_gp: bool = False, _tail_split: int = 16,
                 gr_pe: bool = False, w_bufs: int = 2,
                 _skip: str = '') -> bass.Bass:
    if blob:
        assert fp8_pot and fp8_dep, "blob layout assumes fp8 DA tensors" 
    # Bacc (not plain Bass): its finalize() runs move_matmul_waits_to_
    # ldweights + generate_event_semaphores, which split multi-sem waits to
    # satisfy the 1-wait-per-instruction TRN2 codegen constraint.
    nc = bacc.Bacc()
    dt = mybir.dt
    f32, bf16 = dt.float32, dt.bfloat16
    dt_p = dt.float8e4 if fp8_pot else bf16
    dt_d = dt.float8e4 if fp8_dep else bf16

    # Per-core inputs (host pre-packed into DMA-friendly layouts).
    if blob:
        # One byte-interleaved tensor per super: per partition row =
        # [DA_p h0 | DA_p h1 | DA_d h0 | DA_d h1 (fp8, N bytes each) | W (f32, 4N bytes)]
        BLOB = nc.dram_tensor("BLOB", [NSUP, 128, 8 * N], dt.uint8,
                              kind="ExternalInput")
    else:
        DA_p = nc.dram_tensor("DA_p", [NSUP, 2, 128, N], dt_p, kind="ExternalInput")
        DA_d = nc.dram_tensor("DA_d", [NSUP, 2, 128, N], dt_d, kind="ExternalInput")
        W_in = nc.dram_tensor("W_in", [NSUP, 128, N], f32, kind="ExternalInput")
    LHS_p = nc.dram_tensor("LHS_p", [128, 2 * NSUP * 64], bf16, kind="ExternalInput")
    dt_ld = dt.float8e4 if lhsd_fp8 else bf16
    LHS_d = nc.dram_tensor("LHS_d", [128, 2 * NSUP * 64], dt_ld, kind="ExternalInput")
    if gr_pe:
        GRC_p = nc.dram_tensor("GRC_p", [8, N], bf16, kind="ExternalInput")
        GRC_d = nc.dram_tensor("GRC_d", [8, N], bf16, kind="ExternalInput")
        REP = nc.dram_tensor("REP", [8, 128], bf16, kind="ExternalInput")
    else:
        GR_p = nc.dram_tensor("GR_p", [128, N], bf16, kind="ExternalInput")
        GR_d = nc.dram_tensor("GR_d", [128, N], bf16, kind="ExternalInput")
    XB = nc.dram_tensor("XB", [128, E], f32, kind="ExternalInput")      # xbar slice
    XDEC = nc.dram_tensor("XDEC", [128, E], f32, kind="ExternalInput")  # (1-ax)*Xd
    # Per-core o-slice of the u/V traces, packed column-wise on partitions
    # 0:8 (engine APs must start at partition 0/32/64/96):
    # cols = [u_pot | (1-ap)*V | u_dep | (1-ad)*V], each E wide.
    UV = nc.dram_tensor("UV", [8, 4 * E], f32, kind="ExternalInput")

    W_new = nc.dram_tensor("W_new", [NSUP, 128, N], f32, kind="ExternalOutput")
    XB_new = nc.dram_tensor("XB_new", [128, E], f32, kind="ExternalOutput")
    UVN = nc.dram_tensor("UVN", [8, 2 * E], f32, kind="ExternalOutput")

    mul = mybir.AluOpType.mult
    add = mybir.AluOpType.add
    sub = mybir.AluOpType.subtract
    amin = mybir.AluOpType.min
    amax = mybir.AluOpType.max

    with TileContext(nc) as tc:
        with (
            tc.tile_pool(name="const", bufs=1) as cpool,
            tc.tile_pool(name="rhs", bufs=rhs_bufs) as rhs_pool,
            tc.tile_pool(name="wio", bufs=w_bufs) as w_pool,
            tc.tile_pool(name="ev", bufs=ev_bufs) as ev_pool,
            # wide mode uses (128, 1024) = 2-bank PSUM tiles; 2 tags x 2 bufs
            # x 2 banks = all 8 banks.
            tc.tile_pool(name="psum",
                         bufs=(2 if ev_mode in ('wide', 'wide2') else psum_bufs),
                         space="PSUM") as psum_pool,
        ):
            # One-time constant loads.  In 'wide2' mode the slices needed by
            # the first super load first as separate small tiles so the first
            # compute chain starts after ~200 KB instead of ~2.2 MB.
            fine = ev_mode == 'wide2'
            if fine:
                lhs_p0 = cpool.tile([128, 128], bf16, tag="lhs_p0")
                nc.scalar.dma_start(lhs_p0[:], LHS_p[:, 0:128])
                lhs_d0 = cpool.tile([128, 128], dt_ld, tag="lhs_d0")
                nc.scalar.dma_start(lhs_d0[:], LHS_d[:, 0:128])
                gr_p0 = cpool.tile([128, 1024], bf16, tag="gr_p0")
                nc.scalar.dma_start(gr_p0[:], GR_p[:, 0:1024])
                gr_d0 = cpool.tile([128, 1024], bf16, tag="gr_d0")
                nc.scalar.dma_start(gr_d0[:], GR_d[:, 0:1024])
            head = False  # head reorder measured neutral; stream-paced
            h_tiles = {}
            if head:
                # Interleave super-0's loads with the constants in dependency
                # order, so the first matmul->evac->gate chain starts after
                # ~1 MB instead of ~2.3 MB of DMA.
                lhs_p = cpool.tile([128, 2 * NSUP * 64], bf16, tag="lhs_p")
                nc.sync.dma_start(lhs_p[:], LHS_p[:])
                h_tiles['rp0'] = rhs_pool.tile([128, N], dt_p, tag="rp0", name="h_rp0")
                nc.sync.dma_start(h_tiles['rp0'][:], DA_p[0, 0])
                h_tiles['rp1'] = rhs_pool.tile([128, N], dt_p, tag="rp1", name="h_rp1")
                nc.sync.dma_start(h_tiles['rp1'][:], DA_p[0, 1])
                gr_p = cpool.tile([128, N], bf16, tag="gr_p")
                nc.sync.dma_start(gr_p[:], GR_p[:])
                lhs_d = cpool.tile([128, 2 * NSUP * 64], dt_ld, tag="lhs_d")
                nc.sync.dma_start(lhs_d[:], LHS_d[:])
                h_tiles['rd0'] = rhs_pool.tile([128, N], dt_d, tag="rd0", name="h_rd0")
                nc.sync.dma_start(h_tiles['rd0'][:], DA_d[0, 0])
                h_tiles['rd1'] = rhs_pool.tile([128, N], dt_d, tag="rd1", name="h_rd1")
                nc.sync.dma_start(h_tiles['rd1'][:], DA_d[0, 1])
                gr_d = cpool.tile([128, N], bf16, tag="gr_d")
                nc.sync.dma_start(gr_d[:], GR_d[:])
                h_tiles['wt'] = w_pool.tile([128, N], f32, tag="wt", name="h_wt")
                nc.sync.dma_start(h_tiles['wt'][:], W_in[0])
            else:
                lhs_p = cpool.tile([128, 2 * NSUP * 64], bf16, tag="lhs_p")
                nc.sync.dma_start(lhs_p[:], LHS_p[:])
                lhs_d = cpool.tile([128, 2 * NSUP * 64], dt_ld, tag="lhs_d")
                nc.sync.dma_start(lhs_d[:], LHS_d[:])
                gr_p = cpool.tile([128, N], bf16, tag="gr_p")
                gr_d = cpool.tile([128, N], bf16, tag="gr_d")
                if gr_pe:
                    # Build the partition-replicated gate tensors on-chip:
                    # PE multiplies the compact (8, N) rows by a 0/1 selector
                    # (exact, x1.0 accumulate) - saves ~1 MB of HBM traffic.
                    grc_p = cpool.tile([8, N], bf16, tag="grc_p")
                    nc.scalar.dma_start(grc_p[:], GRC_p[:])
                    grc_d = cpool.tile([8, N], bf16, tag="grc_d")
                    nc.scalar.dma_start(grc_d[:], GRC_d[:])
                    rept = cpool.tile([8, 128], bf16, tag="rept")
                    nc.scalar.dma_start(rept[:], REP[:])
                    for gsrc, gdst, ptag in ((grc_p, gr_p, "pp"), (grc_d, gr_d, "pd")):
                        for h in range(N // 1024):
                            rt = psum_pool.tile([128, 1024], f32, tag=ptag)
                            for q in range(2):
                                c0 = h * 1024 + q * 512
                                nc.tensor.matmul(rt[:, q * 512:(q + 1) * 512],
                                                 rept[:], gsrc[:, c0:c0 + 512],
                                                 start=True, stop=True)
                            nc.scalar.copy(gdst[:, h * 1024:(h + 1) * 1024], rt[:])
                else:
                    # gate tensors via the ACT ring: it is store-only otherwise,
                    # so its sequencer is idle at t=0 and the transfers overlap
                    # the SP ring's constant loads.
                    nc.scalar.dma_start(gr_p[:], GR_p[:])
                    nc.scalar.dma_start(gr_d[:], GR_d[:])

            def emit_traces():
                # Trace updates (tiny).
                xb = ev_pool.tile([128, E], f32, tag="xb")
                nc.sync.dma_start(xb[:], XB[:])
                xd = ev_pool.tile([128, E], f32, tag="xd")
                nc.sync.dma_start(xd[:], XDEC[:])
                xn = ev_pool.tile([128, E], f32, tag="xn")
                nc.vector.scalar_tensor_tensor(xn[:], xb[:], ALPHA_X, xd[:], mul, add)
                nc.sync.dma_start(XB_new[:], xn[:])

                uv = ev_pool.tile([8, 4 * E], f32, tag="uv")
                nc.sync.dma_start(uv[:], UV[:])
                uvn = ev_pool.tile([8, 2 * E], f32, tag="uvn")
                nc.vector.scalar_tensor_tensor(uvn[:, 0:E], uv[:, 0:E],
                                               ALPHA_P, uv[:, E:2 * E], mul, add)
                nc.vector.scalar_tensor_tensor(uvn[:, E:2 * E], uv[:, 2 * E:3 * E],
                                               ALPHA_D, uv[:, 3 * E:4 * E], mul, add)
                nc.sync.dma_start(UVN[:], uvn[:])

            for r in range(repeat):
                for s in range(NSUP):
                    if head and r == 0 and s == 0:
                        rp0, rp1 = h_tiles['rp0'], h_tiles['rp1']
                        rd0, rd1 = h_tiles['rd0'], h_tiles['rd1']
                        wt = h_tiles['wt']
                    elif ev_mode == 'wide2' and s == 0:
                        rp0 = rp1 = rd0 = rd1 = wt = None  # fine-grained below
                    elif blob:
                        bt = rhs_pool.tile([128, 8 * N], dt.uint8, tag="bt")
                        nc.sync.dma_start(bt[:], BLOB[s])
                        rp0 = bt[:, 0 * N:1 * N].bitcast(dt_p)
                        rp1 = bt[:, 1 * N:2 * N].bitcast(dt_p)
                        rd0 = bt[:, 2 * N:3 * N].bitcast(dt_d)
                        rd1 = bt[:, 3 * N:4 * N].bitcast(dt_d)
                        wt = bt[:, 4 * N:8 * N].bitcast(f32)
                    else:
                        rp0 = rhs_pool.tile([128, N], dt_p, tag="rp0")
                        rp1 = rhs_pool.tile([128, N], dt_p, tag="rp1")
                        rd0 = rhs_pool.tile([128, N], dt_d, tag="rd0")
                        rd1 = rhs_pool.tile([128, N], dt_d, tag="rd1")
                        wt = w_pool.tile([128, N], f32, tag="wt")
                        if dma_split:
                            h = N // 2
                            for t_, src in ((rp0, DA_p[s, 0]), (rp1, DA_p[s, 1]),
                                            (rd0, DA_d[s, 0]), (rd1, DA_d[s, 1]),
                                            (wt, W_in[s])):
                                nc.sync.dma_start(t_[:, 0:h], src[:, 0:h])
                                nc.sync.dma_start(t_[:, h:N], src[:, h:N])
                        else:
                            nc.sync.dma_start(rp0[:], DA_p[s, 0])
                            nc.sync.dma_start(rp1[:], DA_p[s, 1])
                            rd_eng = nc.gpsimd if rd_gp else (nc.scalar if rd_act else nc.sync)
                            rd_eng.dma_start(rd0[:], DA_d[s, 0])
                            rd_eng.dma_start(rd1[:], DA_d[s, 1])
                            (nc.gpsimd if w_gp else (nc.scalar if w_act else nc.sync)).dma_start(wt[:], W_in[s])
                    wo = w_pool.tile([128, N], f32, tag="wo")

                    j0, j1 = 2 * s, 2 * s + 1
                    if _skip == 'dmaonly':
                        nc.vector.tensor_copy(wo[:, 0:512], wt[:, 0:512])
                        nc.sync.dma_start(W_new[s], wo[:])
                        continue
                    if ev_mode == 'wide2' and s == 0:
                        # First super at oc granularity with per-chunk DMAs
                        # into separate tiles: the first matmul/DVE chain
                        # starts as soon as its ~200 KB lands.
                        for oc in range(OC):
                            osl = bass.ts(oc, 512)
                            fr0 = rhs_pool.tile([128, 512], dt_p, tag="f0")
                            fr1 = rhs_pool.tile([128, 512], dt_p, tag="f1")
                            fr2 = rhs_pool.tile([128, 512], dt_d, tag="f2")
                            fr3 = rhs_pool.tile([128, 512], dt_d, tag="f3")
                            nc.scalar.dma_start(fr0[:], DA_p[0, 0][:, osl])
                            nc.scalar.dma_start(fr1[:], DA_p[0, 1][:, osl])
                            nc.scalar.dma_start(fr2[:], DA_d[0, 0][:, osl])
                            nc.scalar.dma_start(fr3[:], DA_d[0, 1][:, osl])
                            fw = rhs_pool.tile([128, 512], f32, tag="fw")
                            nc.scalar.dma_start(fw[:], W_in[0][:, osl])
                            pp = psum_pool.tile([128, 512], f32, tag="pp")
                            nc.tensor.matmul(pp[0:64, :], lhs_p0[:, 0:64],
                                             fr0[:], start=True, stop=True)
                            nc.tensor.matmul(pp[64:128, :], lhs_p0[:, 64:128],
                                             fr1[:], start=True, stop=True)
                            pd = psum_pool.tile([128, 512], f32, tag="pd")
                            nc.tensor.matmul(pd[0:64, :], lhs_d0[:, 0:64],
                                             fr2[:], start=True, stop=True)
                            nc.tensor.matmul(pd[64:128, :], lhs_d0[:, 64:128],
                                             fr3[:], start=True, stop=True)
                            sp = ev_pool.tile([128, 512], bf16, tag="fsp")
                            nc.scalar.copy(sp[:], pp[:])
                            sd = ev_pool.tile([128, 512], bf16, tag="fsd")
                            nc.scalar.copy(sd[:], pd[:])
                            gp_src = gr_p0 if oc < 2 else gr_p
                            gd_src = gr_d0 if oc < 2 else gr_d
                            t1 = ev_pool.tile([128, 512], bf16, tag="ft1")
                            nc.vector.tensor_tensor(t1[:], sp[:], gp_src[:, osl], mul)
                            t2 = ev_pool.tile([128, 512], bf16, tag="ft2")
                            nc.vector.tensor_tensor(t2[:], sd[:], gd_src[:, osl], mul)
                            t3 = ev_pool.tile([128, 512], f32, tag="ft3")
                            nc.vector.tensor_tensor(t3[:], t1[:], t2[:], sub)
                            t4 = ev_pool.tile([128, 512], f32, tag="ft4")
                            nc.vector.tensor_tensor(t4[:], t3[:], fw[:], add)
                            nc.vector.tensor_scalar(wo[:, osl], t4[:], WMAX, 0.0,
                                                    amin, amax)
                        (nc.scalar if out_act else nc.sync).dma_start(W_new[0], wo[:])
                        continue
                    narrow_tail = False  # measured worse: narrow DVE ops cost more than the shorter tail chain saves
                    if ev_mode in ('wide', 'wide2'):
                        # oc-pair granularity: (128, 1024) PSUM tiles (2 banks),
                        # one ScalarE evacuation + 1024-wide DVE ops per pair -
                        # amortizes the per-op fixed cost with identical math.
                        last = (ev_mode == 'wide2' and s == NSUP - 1) or \
                               (ev_mode == 'wide' and s >= NSUP - _tail_split)
                        # split@1024 stores release DMA work at finer grain;
                        # best in sweep: split every super
                        for ocp in range(OC // 2):
                            pp = psum_pool.tile([128, 1024], f32, tag="pp")
                            pd = psum_pool.tile([128, 1024], f32, tag="pd")
                            for w in range(2):
                                oc = 2 * ocp + w
                                osl = bass.ts(oc, 512)
                                wsl = slice(w * 512, (w + 1) * 512)
                                nc.tensor.matmul(pp[0:64, wsl], lhs_p[:, bass.ts(j0, 64)],
                                                 rp0[:, osl], start=True, stop=True)
                                nc.tensor.matmul(pp[64:128, wsl], lhs_p[:, bass.ts(j1, 64)],
                                                 rp1[:, osl], start=True, stop=True)
                                nc.tensor.matmul(pd[0:64, wsl], lhs_d[:, bass.ts(j0, 64)],
                                                 rd0[:, osl], start=True, stop=True)
                                nc.tensor.matmul(pd[64:128, wsl], lhs_d[:, bass.ts(j1, 64)],
                                                 rd1[:, osl], start=True, stop=True)
                            psl = bass.ts(ocp, 1024)
                            sp = ev_pool.tile([128, 1024], bf16, tag="sp")
                            nc.scalar.copy(sp[:], pp[:])
                            sd = ev_pool.tile([128, 1024], bf16, tag="sd")
                            nc.scalar.copy(sd[:], pd[:])
                            t1 = ev_pool.tile([128, 1024], bf16, tag="t1")
                            nc.vector.tensor_tensor(t1[:], sp[:], gr_p[:, psl], mul)
                            t2 = ev_pool.tile([128, 1024], bf16, tag="t2")
                            nc.vector.tensor_tensor(t2[:], sd[:], gr_d[:, psl], mul)
                            t3 = ev_pool.tile([128, 1024], f32, tag="t3")
                            nc.vector.tensor_tensor(t3[:], t1[:], t2[:], sub)
                            t4 = ev_pool.tile([128, 1024], f32, tag="t4")
                            nc.vector.tensor_tensor(t4[:], t3[:], wt[:, psl], add)
                            nc.vector.tensor_scalar(wo[:, psl], t4[:], WMAX, 0.0,
                                                    amin, amax)
                            if last:
                                # split stores so the final chunk's store is
                                # the only thing on the kernel tail
                                (nc.scalar if out_act else nc.sync).dma_start(
                                    W_new[s][:, psl], wo[:, psl])
                        if not last:
                            (nc.scalar if out_act else nc.sync).dma_start(
                                W_new[s], wo[:])
                        continue
                    for oc in range(OC):
                        osl = bass.ts(oc, 512)
                        pp = psum_pool.tile([128, 512], f32, tag="pp")
                        nc.tensor.matmul(pp[0:64, :], lhs_p[:, bass.ts(j0, 64)],
                                         rp0[:, osl], start=True, stop=True)
                        nc.tensor.matmul(pp[64:128, :], lhs_p[:, bass.ts(j1, 64)],
                                         rp1[:, osl], start=True, stop=True)
                        pd = psum_pool.tile([128, 512], f32, tag="pd")
                        nc.tensor.matmul(pd[0:64, :], lhs_d[:, bass.ts(j0, 64)],
                                         rd0[:, osl], start=True, stop=True)
                        nc.tensor.matmul(pd[64:128, :], lhs_d[:, bass.ts(j1, 64)],
                                         rd1[:, osl], start=True, stop=True)

                        if _skip == 'noev':
                            if oc == 0:
                                nc.vector.tensor_copy(wo[:, 0:512], pp[:])
                            continue
                        sp = ev_pool.tile([128, 512], bf16, tag="sp")
                        nc.scalar.copy(sp[:], pp[:])
                        sd = ev_pool.tile([128, 512], bf16, tag="sd")
                        nc.scalar.copy(sd[:], pd[:])

                        t1 = ev_pool.tile([128, 512], bf16, tag="t1")
                        nc.vector.tensor_tensor(t1[:], sp[:], gr_p[:, osl], mul)
                        t2 = ev_pool.tile([128, 512], bf16, tag="t2")
                        nc.vector.tensor_tensor(t2[:], sd[:], gr_d[:, osl], mul)
                        t3 = ev_pool.tile([128, 512], f32, tag="t3")
                        if ev_mode == 'subvec':
                            nc.vector.tensor_tensor(t3[:], t1[:], t2[:], sub)
                        else:
                            nc.gpsimd.tensor_tensor(t3[:], t1[:], t2[:], sub)
                        t4 = ev_pool.tile([128, 512], f32, tag="t4")
                        if ev_mode == 'v4gp':
                            nc.gpsimd.tensor_tensor(t4[:], t3[:], wt[:, osl], add)
                        else:
                            nc.vector.tensor_tensor(t4[:], t3[:], wt[:, osl], add)
                        if ev_mode == 'clip_gp':
                            nc.gpsimd.tensor_scalar(wo[:, osl], t4[:], WMAX, 0.0,
                                                    amin, amax)
                        else:
                            nc.vector.tensor_scalar(wo[:, osl], t4[:], WMAX, 0.0,
                                                    amin, amax)
                    (nc.scalar if out_act else nc.sync).dma_start(W_new[s], wo[:])
                emit_traces()

    nc.finalize()
    return nc


def prepare_inputs(Xd, Xpost, Vpost, xbar_pre, u_pot, u_dep, W, A_p, A_d, dmap,
                   fp8_pot=False, fp8_dep=False, blob=False, lhsd_fp8=False,
                   gr_pe=False):
    """Host-side shard + repack.  Returns list of per-core input dicts.

    With fp8_*: the folded dmap*A tensor is stored as float8_e4m3 scaled up
    by FP8_SCALE (A ~ 1e-3 underflows e4m3 subnormals); the matching
    replicated gate tensor is scaled down by 1/FP8_SCALE to compensate
    (exact power-of-two, no precision loss).
    """
    gp = (Xpost * np.maximum(u_pot, 0.0)).astype(F32)        # (B, N)
    gd = np.maximum(u_dep, 0.0).astype(F32)
    FP8 = ml_dtypes.float8_e4m3
    sp = 1.0 / FP8_SCALE if fp8_pot else 1.0
    sd = 1.0 / FP8_SCALE if fp8_dep else 1.0
    if gr_pe:
        grc_p = (gp * sp).astype(BF16)                    # (8, N)
        grc_d = (gd * sd).astype(BF16)
        repm = np.zeros((8, 128), dtype=BF16)
        repm[(np.arange(128) % 64) // 8, np.arange(128)] = 1
        gr_io = dict(GRC_p=grc_p, GRC_d=grc_d, REP=repm)
    else:
        grep_p = np.tile(np.repeat(gp * sp, 8, axis=0), (2, 1)).astype(BF16)
        grep_d = np.tile(np.repeat(gd * sd, 8, axis=0), (2, 1)).astype(BF16)
        gr_io = dict(GR_p=grep_p, GR_d=grep_d)

    if fp8_pot:
        DA_p_full = (dmap * (A_p * FP8_SCALE)[None]).astype(FP8)
    else:
        DA_p_full = (dmap * A_p[None]).astype(BF16)          # (D, N, N)
    if fp8_dep:
        DA_d_full = (dmap * (A_d * FP8_SCALE)[None]).astype(FP8)
    else:
        DA_d_full = (dmap * A_d[None]).astype(BF16)

    xb_flat = xbar_pre.reshape(D * B, N)
    xdec_flat = (np.float32(1.0 - ALPHA_X) * Xd).reshape(D * B, N).astype(F32)
    vp1 = (np.float32(1.0 - ALPHA_P) * Vpost).astype(F32)
    vp2 = (np.float32(1.0 - ALPHA_D) * Vpost).astype(F32)

    def pack_lhs(src, dtype):
        # src: (D, B, N) -> per-core (128, 2*NSUP*64) block-diagonal lhsT.
        out = []
        for ci in range(NCORES):
            sl = slice(ci * E, (ci + 1) * E)
            xs = src[:, :, sl].reshape(D, B, 2 * NSUP, 8)    # d, b, j, el
            blk = np.zeros((2 * NSUP, 8, D, B, 8), dtype=F32)  # j, el_k, d, b, el_m
            for el in range(8):
                blk[:, el, :, :, el] = xs[:, :, :, el].transpose(2, 0, 1)
            # k = el*16 + d (el outer), col = j*64 + b*8 + el_m
            lhs = blk.reshape(2 * NSUP, 128, 64).transpose(1, 0, 2).reshape(128, -1)
            out.append(np.ascontiguousarray(lhs).astype(dtype))
        return out

    lhs_p_cores = pack_lhs(xbar_pre, BF16)
    # Xd is binary 0/1 - exact in fp8, halves the lhsT bytes.
    lhs_d_cores = pack_lhs(Xd, FP8 if lhsd_fp8 else BF16)

    in_maps = []
    for ci in range(NCORES):
        sl = slice(ci * E, (ci + 1) * E)

        def pack_da(full):
            # (D, E, N) -> (NSUP, 2, 128, N), k = el*16 + d
            x = full[:, sl].reshape(D, NSUP, 2, 8, N).transpose(1, 2, 3, 0, 4)
            return np.ascontiguousarray(x.reshape(NSUP, 2, 128, N))

        w_c = W[:, sl].reshape(B, NSUP, 2, 8, N).transpose(1, 2, 0, 3, 4)
        w_c = np.ascontiguousarray(w_c.reshape(NSUP, 128, N), dtype=F32)

        if blob:
            pda_p = pack_da(DA_p_full)  # (NSUP, 2, 128, N) fp8
            pda_d = pack_da(DA_d_full)
            bl = np.empty((NSUP, 128, 8 * N), np.uint8)
            bl[:, :, 0 * N:1 * N] = pda_p[:, 0].view(np.uint8)
            bl[:, :, 1 * N:2 * N] = pda_p[:, 1].view(np.uint8)
            bl[:, :, 2 * N:3 * N] = pda_d[:, 0].view(np.uint8)
            bl[:, :, 3 * N:4 * N] = pda_d[:, 1].view(np.uint8)
            bl[:, :, 4 * N:8 * N] = w_c.view(np.uint8)
            io = dict(BLOB=bl)
        else:
            io = dict(DA_p=pack_da(DA_p_full), DA_d=pack_da(DA_d_full), W_in=w_c)

        in_maps.append(dict(
            **io,
            LHS_p=lhs_p_cores[ci], LHS_d=lhs_d_cores[ci],
            **gr_io,
            XB=np.ascontiguousarray(xb_flat[:, sl], dtype=F32),
            XDEC=np.ascontiguousarray(xdec_flat[:, sl]),
            UV=np.concatenate([u_pot[:, sl], vp1[:, sl],
                               u_dep[:, sl], vp2[:, sl]], axis=1).astype(F32),
        ))
    return in_maps


def assemble_outputs(results, W):
    W_new = np.empty((B, N, N), dtype=F32)
    xbar_new = np.empty((D * B, N), dtype=F32)
    for ci in range(NCORES):
        sl = slice(ci * E, (ci + 1) * E)
        wn = results[ci]["W_new"].reshape(NSUP, 2, B, 8, N).transpose(2, 0, 1, 3, 4)
        W_new[:, sl] = wn.reshape(B, E, N)
        xbar_new[:, sl] = results[ci]["XB_new"]
    u_pot_new = np.empty((B, N), dtype=F32)
    u_dep_new = np.empty((B, N), dtype=F32)
    for ci in range(NCORES):
        sl = slice(ci * E, (ci + 1) * E)
        u_pot_new[:, sl] = results[ci]["UVN"][:, 0:E]
        u_dep_new[:, sl] = results[ci]["UVN"][:, E:2 * E]
    out = np.asarray(W, dtype=F32)
    return out, W_new, xbar_new.reshape(D, B, N), u_pot_new, u_dep_new


# Production configuration: fp8 folded dmap*A tensors (exact 0/1 dmap,
# power-of-two compensation in the gate tensors), all-DVE elementwise,
# W_new stores issued from the ACT HWDGE ring.
KERNEL_CFG = dict(fp8_pot=True, fp8_dep=True, ev_mode="wide", out_act=True,
                  lhsd_fp8=True, gr_pe=True, w_bufs=3)

_NC_CACHE: dict = {}


def _get_nc():
    if "nc" not in _NC_CACHE:
        _NC_CACHE["nc"] = build_kernel(**KERNEL_CFG)
    return _NC_CACHE["nc"]


def kernel(Xd, Xpost, Vpost, xbar_pre, u_pot, u_dep, W, A_p, A_d, dmap):
    args = [np.asarray(a, dtype=F32) for a in
            (Xd, Xpost, Vpost, xbar_pre, u_pot, u_dep, W, A_p, A_d, dmap)]
    in_maps = prepare_inputs(*args, fp8_pot=KERNEL_CFG["fp8_pot"],
                             fp8_dep=KERNEL_CFG["fp8_dep"],
                             lhsd_fp8=KERNEL_CFG["lhsd_fp8"],
                             gr_pe=KERNEL_CFG["gr_pe"])
    nc = _get_nc()
    # The axon-tunneled device occasionally reports a transient
    # NRT_EXEC_UNIT_UNRECOVERABLE that clears after the remote worker
    # restarts; retry a couple of times before giving up.
    last_exc = None
    for attempt in range(3):
        try:
            res = run_bass_kernel_spmd(nc, in_maps, core_ids=list(range(NCORES)))
            return assemble_outputs(res.results, args[6])
        except Exception as exc:  # noqa: BLE001
            last_exc = exc
            if attempt < 2:
                import time
                time.sleep(45)
    raise last_exc
